# revision 1
# baseline (speedup 1.0000x reference)
"""Trainium2 Bass kernel for nn_AttentionWithVQ (B=4, N=2048, DIM=512, H=8,
depthwise-conv positional term, softmax attention, output projection).

Sharding: data-parallel over B (4 batches x 2 core-groups) and tensor-parallel
over heads (4 heads per core) -> 8 cores, fully independent per core except a
final partial-sum over the two head-groups of each batch, done on host at
gather time (the output projection contracts over heads).

Core algorithmic fusion: the score matrix
    S = 0.5*(scale * q @ k^T + scale * conv1(m) @ conv2(s)^T)
is ONE matmul over a concatenated 128-feature axis:
    S = Qp^T @ Kp,  Qp = [q*scale*0.5 ; conv1(m)*scale*0.5], Kp = [k ; conv2(s)]
which exactly fills the 128x128 PE array contraction dim.

Softmax denominators come for free by appending a ones-column to V
(attn@[V|1] yields the row-sums of exp(S) in the last output row); exp() is
numerically safe without max-subtraction for this problem's score magnitudes
(|S| < ~1 given the 0.02-scaled weights).

Partition alignment: compute engines are lane-locked (PSUM partition p ->
SBUF partition p), so per-head feature layouts alternate by head parity
(even heads [qk;conv], odd heads [conv;qk]) making every PSUM->SBUF copy
partition-aligned; the few genuinely shifting copies (odd-head attention
outputs, denominator rows) go through DMA, which can move partitions freely.
All permutation bookkeeping is done host-side in numpy when preparing
per-core inputs.
"""

import os
import sys

sys.path.insert(0, "/opt/trn_rl_repo")

import numpy as np

# ---------------------------------------------------------------- constants
B, N, DIM, HEAD, VQE_K = 4, 2048, 512, 8, 3
Dh = DIM // HEAD            # 64
HPC = HEAD // 2             # heads per core (8 cores = 4 batch * 2 groups)
P = 128
FB = 512                    # free-dim block (one fp32 PSUM bank)
NQB = N // FB               # 4
NKB = N // P                # 16
SCALE_Q = Dh ** -0.5 * 0.5  # folds the 0.5 score scale into the q/conv1 side

# which matmul groups run in float32r (1 cyc/row) vs float32 (4 cyc/row)
_DEFAULT_CFG = {"qkv": "f32", "attn": "f32", "proj": "f32"}

_CACHE = {}


# ---------------------------------------------------------------- host prep
def _host_prep(core, inp):
    """Build the per-core input arrays (sharding + layout permutations)."""
    b, g = core // 2, core % 2
    f32 = np.float32
    x, m, s = inp["x"], inp["m"], inp["s"]
    qkv_w, qkv_b = inp["qkv_w"], inp["qkv_b"]
    proj_w = inp["proj_w"]
    p1w = inp["pe1_w"].reshape(HEAD, VQE_K)
    p2w = inp["pe2_w"].reshape(HEAD, VQE_K)
    pe1_b, pe2_b = inp["pe1_b"], inp["pe2_b"]

    d = {}
    d["xt"] = np.ascontiguousarray(x[b].T, dtype=f32)  # [512, 2048]

    # m/s transposed, tile t rows = [head(2t+1) feats ; head(2t) feats]
    mt = np.empty((256, N), f32)
    st = np.empty((256, N), f32)
    mcw = np.zeros((128, 8), f32)
    scw = np.zeros((128, 8), f32)
    for t in range(2):
        h_lo, h_hi = g * 4 + 2 * t + 1, g * 4 + 2 * t
        mt[t * 128:t * 128 + 64] = m[b][:, h_lo * 64:(h_lo + 1) * 64].T
        mt[t * 128 + 64:t * 128 + 128] = m[b][:, h_hi * 64:(h_hi + 1) * 64].T
        st[t * 128:t * 128 + 64] = s[b][:, h_lo * 64:(h_lo + 1) * 64].T
        st[t * 128 + 64:t * 128 + 128] = s[b][:, h_hi * 64:(h_hi + 1) * 64].T
        for p in range(128):
            h = g * 4 + 2 * t + (1 if p < 64 else 0)
            mcw[p, 4 * t:4 * t + 3] = p1w[h] * SCALE_Q
            scw[p, 4 * t:4 * t + 3] = p2w[h]
            mcw[p, 4 * t + 3] = pe1_b[h] * SCALE_Q
            scw[p, 4 * t + 3] = pe2_b[h]
    d["mt"], d["st"], d["mcw"], d["scw"] = mt, st, mcw, scw

    # q/k projection weights: chunk ch=(t, q|k) = [even-head rows; odd-head rows]
    wqk_f = np.empty((512, DIM), f32)
    qkb = np.zeros((128, 4), f32)
    for t in range(2):
        for j in range(2):  # 0=q, 1=k
            ch = 2 * t + j
            h_e, h_o = g * 4 + 2 * t, g * 4 + 2 * t + 1
            base = j * DIM
            wqk_f[ch * 128:ch * 128 + 64] = qkv_w[base + h_e * 64:base + (h_e + 1) * 64]
            wqk_f[ch * 128 + 64:(ch + 1) * 128] = qkv_w[base + h_o * 64:base + (h_o + 1) * 64]
            qkb[0:64, ch] = qkv_b[base + h_e * 64:base + (h_e + 1) * 64]
            qkb[64:128, ch] = qkv_b[base + h_o * 64:base + (h_o + 1) * 64]
            if j == 0:
                wqk_f[ch * 128:(ch + 1) * 128] *= SCALE_Q
                qkb[:, ch] *= SCALE_Q
    d["wqk"] = np.ascontiguousarray(wqk_f.T)  # [c=512, f=512]
    d["qkb"] = qkb

    d["wv"] = np.ascontiguousarray(
        qkv_w[2 * DIM + g * 256:2 * DIM + (g + 1) * 256].T, dtype=f32)  # [512, 256]

    # proj rows / v-bias / one-hot broadcast matrix in aT partition order:
    # aT tile t partition p -> head 2t+(p>=64), d=p%64
    pjt = np.empty((256, DIM), f32)
    vbv = np.empty((256,), f32)
    for t in range(2):
        for p in range(128):
            h_l = 2 * t + (1 if p >= 64 else 0)
            h = g * 4 + h_l
            pjt[t * 128 + p] = proj_w[:, h * 64 + (p % 64)]
            vbv[t * 128 + p] = qkv_b[2 * DIM + h * 64 + (p % 64)]
    d["pjt"] = pjt
    d["vbv"] = np.ascontiguousarray(vbv.reshape(2, 128).T)  # [128, 2]
    return d


# ------------------------------------------------------------- device build
def _emit(tc, nc, io, cfg):
    from contextlib import ExitStack

    from concourse import mybir

    dt = mybir.dt
    f32 = dt.float32
    AF = mybir.ActivationFunctionType
    ALU = mybir.AluOpType

    # float32r tiles must be *produced* as float32r (the BIR verifier
    # requires producer-side rounding), so the dtype is set on the tiles
    # themselves rather than bitcast at the matmul call sites.
    def _dt(v):
        return {"f32": f32, "f32r": dt.float32r, "bf16": dt.bfloat16}[v]

    dt_qkv = _dt(cfg["qkv"])
    dt_attn = _dt(cfg["attn"])
    dt_proj = _dt(cfg["proj"])
    # exp granularity: one ACT op per FBS-wide stripe (matmuls within are
    # still 512-wide: a matmul output cannot cross a PSUM bank)
    FBS = 1024 if dt_attn == dt.bfloat16 else 512
    FBQ = 1024 if dt_qkv == dt.bfloat16 else 512
    NIH = FBS // FB

    with ExitStack() as ctx:
        persist = ctx.enter_context(tc.tile_pool(name="persist", bufs=1))

        # ---- persistent weight / activation tiles
        wqk_sb, wv_sb, xt_sb = [], [], []
        QP, KP, v_sb, aT, pjt_sb, bcsb = [], [], [], [], [], []
        for c in range(4):
            w = persist.tile([128, 512], dt_qkv, name=f"wqk{c}", tag=f"wqk{c}")
            nc.sync.dma_start(w[:], io["wqk"][c * 128:(c + 1) * 128, :])
            wqk_sb.append(w)
        mcw_sb = persist.tile([128, 8], f32, name="mcw", tag="mcw")
        nc.gpsimd.dma_start(mcw_sb[:], io["mcw"][:, :])
        scw_sb = persist.tile([128, 8], f32, name="scw", tag="scw")
        nc.gpsimd.dma_start(scw_sb[:], io["scw"][:, :])
        qkb_sb = persist.tile([128, 4], f32, name="qkb", tag="qkb")
        nc.sync.dma_start(qkb_sb[:], io["qkb"][:, :])

        for h in range(HPC):
            QP.append(persist.tile([128, N], dt_attn, name=f"QP{h}", tag=f"QP{h}"))
            KP.append(persist.tile([128, N], dt_attn, name=f"KP{h}", tag=f"KP{h}"))
        # per-head V block is [v(64) | ones | zero-pad] = 66 columns: matmul
        # operands need 4-byte-aligned offsets, so the block width must be
        # even for 2-byte dtypes (66*h*2 is always 4-aligned)
        for blk in range(NKB):
            v_sb.append(persist.tile([128, HPC * 66], dt_attn, name=f"vsb{blk}",
                                     tag=f"vsb{blk}"))
        for t in range(2):
            aT.append(persist.tile([128, N], dt_proj, name=f"aT{t}", tag=f"aT{t}"))
            bcsb.append(persist.tile([128, N], f32, name=f"bcsb{t}",
                                     tag=f"bcsb{t}"))
        # softmax denominators, DMA-reshaped onto all 128 partitions so the
        # reciprocal runs 32x wider than a [4, N] row layout would allow
        denR = persist.tile([128, N // 32], f32, name="denR", tag="denR")

        # ---- depthwise convs (DVE; emitted first so they overlap the qkv
        # matmuls — conv inputs stream on the gpsimd DMA queue)
        with tc.tile_pool(name="conv", bufs=2) as convp:
            for src, wv_, dst in (("mt", mcw_sb, QP), ("st", scw_sb, KP)):
                for t in range(2):
                    xin = convp.tile([128, N], f32, name=f"ci_{src}{t}", tag="cin")
                    nc.gpsimd.dma_start(xin[:], io[src][t * 128:(t + 1) * 128, :])
                    y = convp.tile([128, N], f32, name=f"cy_{src}{t}", tag="cy",
                                   bufs=1)
                    w0, w1, w2, cb = (wv_[:, 4 * t + k:4 * t + k + 1]
                                      for k in range(4))
                    nc.vector.tensor_scalar(y[:], xin[:], w1, cb,
                                            ALU.mult, ALU.add)
                    nc.vector.scalar_tensor_tensor(
                        y[:, 1:], xin[:, :N - 1], w0, y[:, 1:],
                        ALU.mult, ALU.add)
                    nc.vector.scalar_tensor_tensor(
                        y[:, :N - 1], xin[:, 1:], w2, y[:, :N - 1],
                        ALU.mult, ALU.add)
                    nc.vector.tensor_copy(dst[2 * t + 1][0:64, :], y[0:64, :])
                    nc.vector.tensor_copy(dst[2 * t][64:128, :], y[64:128, :])

        # ---- qkv projections (x^T resident only here)
        with tc.tile_pool(name="xtp", bufs=1) as xtp:
            for c in range(4):
                xt = xtp.tile([128, N], dt_qkv, name=f"xt{c}", tag=f"xt{c}")
                nc.sync.dma_start(xt[:], io["xt"][c * 128:(c + 1) * 128, :])
                xt_sb.append(xt)
            for c in range(4):
                w = persist.tile([128, 256], dt_qkv, name=f"wv{c}", tag=f"wv{c}")
                nc.sync.dma_start(w[:], io["wv"][c * 128:(c + 1) * 128, :])
                wv_sb.append(w)
            vbv_sb = persist.tile([128, 2], f32, name="vbv", tag="vbv")
            nc.sync.dma_start(vbv_sb[:], io["vbv"][:, :])
            for f in range(2):
                w = persist.tile([128, 512], dt_proj, name=f"pjt{f}", tag=f"pjt{f}")
                nc.sync.dma_start(w[:], io["pjt"][f * 128:(f + 1) * 128, :])
                pjt_sb.append(w)

            with tc.tile_pool(name="ps_qkv", bufs=1, space="PSUM") as ps_qkp:
                for t in range(2):
                    for j in range(2):
                        ch = 2 * t + j
                        dst = QP if j == 0 else KP
                        for qb in range(N // FBQ):
                            qs = slice(qb * FBQ, (qb + 1) * FBQ)
                            ps = ps_qkp.tile([128, FBQ], f32, name="psqk",
                                             tag="psqk", bufs=3)
                            for ih in range(FBQ // FB):
                                hqs = slice(qb * FBQ + ih * FB,
                                            qb * FBQ + (ih + 1) * FB)
                                for c in range(4):
                                    nc.tensor.matmul(
                                        ps[:, ih * FB:(ih + 1) * FB],
                                        wqk_sb[c][:, ch * 128:(ch + 1) * 128],
                                        xt_sb[c][:, hqs],
                                        start=(c == 0), stop=(c == 3))
                            nc.vector.tensor_scalar_add(
                                dst[2 * t][0:64, qs], ps[0:64, :],
                                qkb_sb[0:64, ch:ch + 1])
                            nc.vector.tensor_scalar_add(
                                dst[2 * t + 1][64:128, qs], ps[64:128, :],
                                qkb_sb[64:128, ch:ch + 1])
                for blk in range(NKB):
                    bs = slice(blk * 128, (blk + 1) * 128)
                    ps = ps_qkp.tile([128, 256], f32, name="psv", tag="psv",
                                     bufs=2)
                    for c in range(4):
                        nc.tensor.matmul(ps[:], xt_sb[c][:, bs],
                                         wv_sb[c][:],
                                         start=(c == 0), stop=(c == 3))
                    v3 = v_sb[blk].rearrange("p (h f) -> p h f", h=HPC)
                    nc.vector.tensor_copy(v3[:, :, 0:64],
                                          ps.rearrange("p (h f) -> p h f", h=HPC))
                    # memset lacks float32r support; write the ones/pad columns
                    # through an f32 view (identical bit pattern)
                    ones_ap, pad_ap = v3[:, :, 64:65], v3[:, :, 65:66]
                    if dt_attn == dt.float32r:
                        ones_ap = ones_ap.bitcast(f32)
                        pad_ap = pad_ap.bitcast(f32)
                    nc.vector.memset(ones_ap, 1.0)
                    nc.vector.memset(pad_ap, 0.0)

        # ---- attention (fused score matmul + exp + attn@[V|1|0])
        # per-(head, stripe) PSUM output tiles double-buffer so the next
        # group's accumulation starts while the previous one is copied out
        with tc.tile_pool(name="ps_s", bufs=2, space="PSUM") as ps_sp, \
                tc.tile_pool(name="ps_o", bufs=2, space="PSUM") as ps_op, \
                tc.tile_pool(name="esbp", bufs=2) as esbp, \
                tc.tile_pool(name="stg", bufs=2) as stgp:
            for h in range(HPC):
                t, odd = h // 2, h % 2
                vcols = slice(h * 66, (h + 1) * 66)
                for q2 in range(N // FBS):
                    qbase = q2 * FBS
                    cs = slice(qbase, qbase + FBS)
                    o_ps = ps_op.tile([66, FBS], f32, name=f"ops{h}_{q2}",
                                      tag="ops")
                    for nk in range(NKB):
                        ks = slice(nk * 128, (nk + 1) * 128)
                        s_ps = ps_sp.tile([128, FBS], f32, name="sps",
                                          tag="sps")
                        for ih in range(NIH):
                            hqs = slice(qbase + ih * FB, qbase + (ih + 1) * FB)
                            nc.tensor.matmul(s_ps[:, ih * FB:(ih + 1) * FB],
                                             KP[h][:, ks], QP[h][:, hqs],
                                             start=True, stop=True)
                        e_sb = esbp.tile([128, FBS], dt_attn, name="esb",
                                         tag="esb")
                        nc.scalar.activation(e_sb[:], s_ps[:], AF.Exp)
                        for ih in range(NIH):
                            nc.tensor.matmul(
                                o_ps[:, ih * FB:(ih + 1) * FB],
                                v_sb[nk][:, vcols],
                                e_sb[:, ih * FB:(ih + 1) * FB],
                                start=(nk == 0), stop=(nk == NKB - 1))
                    # lane-locked engines cannot shift partitions and DMA
                    # cannot read PSUM, so shifting copies stage through SBUF
                    stgd = stgp.tile([65, FBS], f32, name=f"sd{h}_{q2}",
                                     tag="stgd")
                    if odd:
                        stg = stgp.tile([64, FBS], dt_proj, name=f"sg{h}_{q2}",
                                        tag="stg")
                        nc.vector.tensor_copy(stg[:], o_ps[0:64, :])
                        nc.sync.dma_start(aT[t][64:128, cs], stg[:])
                    else:
                        nc.vector.tensor_copy(aT[t][0:64, cs], o_ps[0:64, :])
                    nc.vector.tensor_copy(stgd[64:65, :], o_ps[64:65, :])
                    # denominator stripe -> denR rows (DMA-reshaped, linear)
                    r0 = h * 32 + q2 * (FBS // 64)
                    nc.sync.dma_start(denR[r0:r0 + FBS // 64, :],
                                      stgd[64:65, :])
                if odd:
                    # both heads of aT[t] done: reciprocal + DMA-replicated
                    # broadcast + normalize, overlapped with later heads
                    nc.vector.reciprocal(denR[t * 64:(t + 1) * 64, :],
                                         denR[t * 64:(t + 1) * 64, :])
                    nc.sync.dma_start(io["drec"][2 * t:2 * t + 2, :],
                                      denR[t * 64:(t + 1) * 64, :])
                    for par in range(2):
                        nc.sync.dma_start(
                            bcsb[t][par * 64:(par + 1) * 64, :],
                            io["drec"][2 * t + par:2 * t + par + 1,
                                       :].broadcast_to([64, N]))
                    nc.vector.tensor_mul(aT[t][:], aT[t][:], bcsb[t][:])
                    nc.vector.tensor_scalar_add(aT[t][:], aT[t][:],
                                                vbv_sb[:, t:t + 1])

        # ---- output projection (partial over this core's heads)
        with tc.tile_pool(name="ps_pj", bufs=3, space="PSUM") as ps_pjp, \
                tc.tile_pool(name="osbp", bufs=3) as osbp:
            for blk in range(NKB):
                bs = slice(blk * 128, (blk + 1) * 128)
                pj = ps_pjp.tile([128, FB], f32, name="pj", tag="pj")
                for f in range(2):
                    nc.tensor.matmul(pj[:], aT[f][:, bs],
                                     pjt_sb[f][:],
                                     start=(f == 0), stop=(f == 1))
                ob = osbp.tile([128, FB], f32, name="ob", tag="ob")
                nc.vector.tensor_copy(ob[:], pj[:])
                nc.gpsimd.dma_start(io["out"][bs, :], ob[:])


def _build(cfg_key):
    from concourse import bacc, mybir, tile

    cfg = dict(cfg_key)
    dt = mybir.dt
    nc = bacc.Bacc("TRN2", target_bir_lowering=False, debug=False,
                   num_devices=8)
    _d = {"f32": dt.float32, "f32r": dt.float32r, "bf16": dt.bfloat16}
    dt_qkv = _d[cfg["qkv"]]
    dt_proj = _d[cfg["proj"]]
    shapes = {
        "xt": ([DIM, N], dt_qkv), "mt": ([256, N], dt.float32),
        "st": ([256, N], dt.float32),
        "wqk": ([DIM, 512], dt_qkv), "wv": ([DIM, 256], dt_qkv),
        "pjt": ([256, DIM], dt_proj),
        "mcw": ([128, 8], dt.float32),
        "scw": ([128, 8], dt.float32),
        "qkb": ([128, 4], dt.float32), "vbv": ([128, 2], dt.float32),
    }
    io = {}
    for name, (shape, dtt) in shapes.items():
        io[name] = nc.dram_tensor(name, shape, dtt,
                                  kind="ExternalInput").ap()
    io["out"] = nc.dram_tensor("out", [N, DIM], dt.float32,
                               kind="ExternalOutput").ap()
    # internal DRAM bounce for the denominator broadcast (DMA cannot
    # replicate from an SBUF source, but a DRAM source AP is linear and
    # supports a zero-step leading dim)
    io["drec"] = nc.dram_tensor("drec", [4, N], dt.float32).ap()
    with tile.TileContext(nc) as tc:
        _emit(tc, nc, io, cfg)
    nc.compile()
    return nc


def _get_program(cfg):
    key = tuple(sorted(cfg.items()))
    if key not in _CACHE:
        _CACHE[key] = _build(key)
    return _CACHE[key]


# ------------------------------------------------------------------ wrapper
def kernel(_cfg=None, _want_results=False, **inputs):
    from concourse.bass_utils import run_bass_kernel_spmd

    cfg = dict(_DEFAULT_CFG)
    if _cfg:
        cfg.update(_cfg)
    env_cfg = os.environ.get("BASSKERN_CFG")
    if env_cfg:  # e.g. "attn=f32r,qkv=f32r"
        for kv in env_cfg.split(","):
            k, v = kv.split("=")
            cfg[k] = v

    inputs = {k: np.asarray(v, dtype=np.float32) for k, v in inputs.items()}
    nc = _get_program(cfg)
    in_maps = [_host_prep(core, inputs) for core in range(8)]
    # bf16 configs declare the corresponding DRAM tensors as bfloat16
    conv_keys = []
    if cfg["qkv"] == "bf16":
        conv_keys += ["xt", "wqk", "wv"]
    if cfg["proj"] == "bf16":
        conv_keys += ["pjt"]
    if conv_keys:
        import ml_dtypes
        for im in in_maps:
            for k in conv_keys:
                im[k] = im[k].astype(ml_dtypes.bfloat16)
    res = run_bass_kernel_spmd(nc, in_maps, list(range(8)))

    out = np.empty((B, N, DIM), np.float32)
    pb = inputs["proj_b"]
    for b in range(B):
        out[b] = res.results[2 * b]["out"] + res.results[2 * b + 1]["out"] + pb
    if _want_results:
        return out, res
    return out



# revision 6
# speedup vs baseline: 1.1573x; 1.1573x over previous
"""Trainium2 Bass kernel for nn_AttentionWithVQ (B=4, N=2048, DIM=512, H=8,
depthwise-conv positional term, softmax attention, output projection).

Sharding: data-parallel over B (4 batches x 2 core-groups) and tensor-parallel
over heads (4 heads per core) -> 8 cores, fully independent per core except a
final partial-sum over the two head-groups of each batch, done on host at
gather time (the output projection contracts over heads).

Core algorithmic fusion: the score matrix
    S = 0.5*(scale * q @ k^T + scale * conv1(m) @ conv2(s)^T)
is ONE matmul over a concatenated 128-feature axis:
    S = Qp^T @ Kp,  Qp = [q*scale*0.5 ; conv1(m)*scale*0.5], Kp = [k ; conv2(s)]
which exactly fills the 128x128 PE array contraction dim.

Softmax denominators come for free by appending a ones-column to V
(attn@[V|1] yields the row-sums of exp(S) in the last output row); exp() is
numerically safe without max-subtraction for this problem's score magnitudes
(|S| < ~1 given the 0.02-scaled weights).

Partition alignment: compute engines are lane-locked (PSUM partition p ->
SBUF partition p), so per-head feature layouts alternate by head parity
(even heads [qk;conv], odd heads [conv;qk]) making every PSUM->SBUF copy
partition-aligned; the few genuinely shifting copies (odd-head attention
outputs, denominator rows) go through DMA, which can move partitions freely.
All permutation bookkeeping is done host-side in numpy when preparing
per-core inputs.

Schedule (v2): the kernel is a single software-pipelined stream ordered to
keep the PE and ACT engines saturated end-to-end:
  - input DMAs are split across the SP/ACT/Pool queues with the
    qkv-critical tensors (xt, wqk) first;
  - qkv bias-adds run on the ACT engine (idle until the first exp);
  - attention runs stripe-outer (q 1024-blocks) / head-inner, with
    scores(nk+1) emitted before attnV(nk) so exp latency is hidden, the
    v-projection matmuls interleaved into the first head's window and the
    second half of the q/k projection into the second head's window;
  - each stripe is normalized, projected, and DMA'd out as soon as its
    4 heads finish, overlapping the next stripe's attention.
"""

import os
import sys

sys.path.insert(0, "/opt/trn_rl_repo")

import numpy as np

# ---------------------------------------------------------------- constants
B, N, DIM, HEAD, VQE_K = 4, 2048, 512, 8, 3
Dh = DIM // HEAD            # 64
HPC = HEAD // 2             # heads per core (8 cores = 4 batch * 2 groups)
P = 128
FB = 512                    # one fp32 PSUM bank
NKB = N // P                # 16
SCALE_Q = Dh ** -0.5 * 0.5  # folds the 0.5 score scale into the q/conv1 side

_DEFAULT_CFG = {"qkv": "bf16", "attn": "bf16", "proj": "bf16"}

_CACHE = {}


def _np_dt(v):
    if v == "bf16":
        import ml_dtypes
        return ml_dtypes.bfloat16
    return np.float32


# ---------------------------------------------------------------- host prep
def _host_prep(core, inp, cfg=None):
    """Build the per-core input arrays (sharding + layout permutations)."""
    cfg = cfg or _DEFAULT_CFG
    b, g = core // 2, core % 2
    f32 = np.float32
    x, m, s = inp["x"], inp["m"], inp["s"]
    qkv_w, qkv_b = inp["qkv_w"], inp["qkv_b"]
    proj_w = inp["proj_w"]
    p1w = inp["pe1_w"].reshape(HEAD, VQE_K)
    p2w = inp["pe2_w"].reshape(HEAD, VQE_K)
    pe1_b, pe2_b = inp["pe1_b"], inp["pe2_b"]
    dt_qkv = _np_dt(cfg["qkv"])
    dt_proj = _np_dt(cfg["proj"])
    dt_conv = _np_dt("bf16" if cfg["attn"] == "bf16" else "f32")

    d = {}
    d["xt"] = np.ascontiguousarray(x[b].T).astype(dt_qkv)  # [512, 2048]

    # m/s transposed, tile t rows = [head(2t+1) feats ; head(2t) feats]
    mt = np.empty((256, N), f32)
    st = np.empty((256, N), f32)
    mcw = np.zeros((128, 8), f32)
    scw = np.zeros((128, 8), f32)
    for t in range(2):
        h_lo, h_hi = g * 4 + 2 * t + 1, g * 4 + 2 * t
        mt[t * 128:t * 128 + 64] = m[b][:, h_lo * 64:(h_lo + 1) * 64].T
        mt[t * 128 + 64:t * 128 + 128] = m[b][:, h_hi * 64:(h_hi + 1) * 64].T
        st[t * 128:t * 128 + 64] = s[b][:, h_lo * 64:(h_lo + 1) * 64].T
        st[t * 128 + 64:t * 128 + 128] = s[b][:, h_hi * 64:(h_hi + 1) * 64].T
        for p in range(128):
            h = g * 4 + 2 * t + (1 if p < 64 else 0)
            mcw[p, 4 * t:4 * t + 3] = p1w[h] * SCALE_Q
            scw[p, 4 * t:4 * t + 3] = p2w[h]
            mcw[p, 4 * t + 3] = pe1_b[h] * SCALE_Q
            scw[p, 4 * t + 3] = pe2_b[h]
    d["mt"], d["st"] = mt.astype(dt_conv), st.astype(dt_conv)
    d["mcw"], d["scw"] = mcw, scw

    # q/k projection weights: chunk ch=(t, q|k) = [even-head rows; odd-head rows]
    wqk_f = np.empty((512, DIM), f32)
    qkb = np.zeros((128, 4), f32)
    for t in range(2):
        for j in range(2):  # 0=q, 1=k
            ch = 2 * t + j
            h_e, h_o = g * 4 + 2 * t, g * 4 + 2 * t + 1
            base = j * DIM
            wqk_f[ch * 128:ch * 128 + 64] = qkv_w[base + h_e * 64:base + (h_e + 1) * 64]
            wqk_f[ch * 128 + 64:(ch + 1) * 128] = qkv_w[base + h_o * 64:base + (h_o + 1) * 64]
            qkb[0:64, ch] = qkv_b[base + h_e * 64:base + (h_e + 1) * 64]
            qkb[64:128, ch] = qkv_b[base + h_o * 64:base + (h_o + 1) * 64]
            if j == 0:
                wqk_f[ch * 128:(ch + 1) * 128] *= SCALE_Q
                qkb[:, ch] *= SCALE_Q
    d["wqk"] = np.ascontiguousarray(wqk_f.T).astype(dt_qkv)  # [c=512, f=512]
    d["qkb"] = qkb

    d["wv"] = np.ascontiguousarray(
        qkv_w[2 * DIM + g * 256:2 * DIM + (g + 1) * 256].T).astype(dt_qkv)

    # proj rows / v-bias in aT partition order:
    # aT tile t partition p -> head 2t+(p>=64), d=p%64
    pjt = np.empty((256, DIM), f32)
    vbv = np.empty((256,), f32)
    for t in range(2):
        for p in range(128):
            h_l = 2 * t + (1 if p >= 64 else 0)
            h = g * 4 + h_l
            pjt[t * 128 + p] = proj_w[:, h * 64 + (p % 64)]
            vbv[t * 128 + p] = qkv_b[2 * DIM + h * 64 + (p % 64)]
    d["pjt"] = pjt.astype(dt_proj)
    d["vbv"] = np.ascontiguousarray(vbv.reshape(2, 128).T.astype(f32))  # [128, 2]
    return d


# ------------------------------------------------------------- device build
def _emit(tc, nc, io, cfg):
    from contextlib import ExitStack

    from concourse import mybir

    dt = mybir.dt
    f32 = dt.float32
    AF = mybir.ActivationFunctionType
    ALU = mybir.AluOpType

    def _dt(v):
        return {"f32": f32, "f32r": dt.float32r, "bf16": dt.bfloat16}[v]

    dt_qkv = _dt(cfg["qkv"])
    dt_attn = _dt(cfg["attn"])
    dt_proj = _dt(cfg["proj"])
    dt_conv = dt.bfloat16 if dt_attn == dt.bfloat16 else f32
    # exp granularity: one ACT op per FBS-wide stripe (matmuls within are
    # still 512-wide: a matmul output cannot cross a PSUM bank)
    FBS = 1024
    NIH = FBS // FB           # 2
    NQ2 = N // FBS            # 2 q-stripes

    with ExitStack() as ctx:
        persist = ctx.enter_context(tc.tile_pool(name="persist", bufs=1))

        # ---- persistent tiles
        wqk_sb, wv_sb, xt_sb = [], [], []
        QP, KP, v_sb, aT, pjt_sb, bcsb = [], [], [], [], [], []
        # critical-path inputs first: xt on the SP queue, wqk on the ACT
        # queue so both stream concurrently from t=0
        for c in range(4):
            xt = persist.tile([128, N], dt_qkv, name=f"xt{c}", tag=f"xt{c}")
            nc.sync.dma_start(xt[:], io["xt"][c * 128:(c + 1) * 128, :])
            xt_sb.append(xt)
        for c in range(4):
            w = persist.tile([128, 512], dt_qkv, name=f"wqk{c}", tag=f"wqk{c}")
            nc.scalar.dma_start(w[:], io["wqk"][c * 128:(c + 1) * 128, :])
            wqk_sb.append(w)
        mcw_sb = persist.tile([128, 8], f32, name="mcw", tag="mcw")
        nc.gpsimd.dma_start(mcw_sb[:], io["mcw"][:, :])
        scw_sb = persist.tile([128, 8], f32, name="scw", tag="scw")
        nc.gpsimd.dma_start(scw_sb[:], io["scw"][:, :])
        qkb_sb = persist.tile([128, 4], f32, name="qkb", tag="qkb")
        nc.sync.dma_start(qkb_sb[:], io["qkb"][:, :])

        for h in range(HPC):
            QP.append(persist.tile([128, N], dt_attn, name=f"QP{h}", tag=f"QP{h}"))
            KP.append(persist.tile([128, N], dt_attn, name=f"KP{h}", tag=f"KP{h}"))
        # per-head V block is [v(64) | ones | zero-pad] = 66 columns: matmul
        # operands need 4-byte-aligned offsets, so the block width must be
        # even for 2-byte dtypes (66*h*2 is always 4-aligned)
        for blk in range(NKB):
            v_sb.append(persist.tile([128, HPC * 66], dt_attn, name=f"vsb{blk}",
                                     tag=f"vsb{blk}"))
        dt_bc = dt.bfloat16 if dt_proj == dt.bfloat16 else f32
        for t in range(2):
            aT.append(persist.tile([128, N], dt_proj, name=f"aT{t}", tag=f"aT{t}"))
            bcsb.append(persist.tile([128, N], dt_bc, name=f"bcsb{t}",
                                     tag=f"bcsb{t}"))
        # softmax denominators, DMA-reshaped onto all 128 partitions so the
        # reciprocal runs 32x wider than a [4, N] row layout would allow
        denR = persist.tile([128, N // 32], f32, name="denR", tag="denR")

        for c in range(4):
            w = persist.tile([128, 256], dt_qkv, name=f"wv{c}", tag=f"wv{c}")
            nc.scalar.dma_start(w[:], io["wv"][c * 128:(c + 1) * 128, :])
            wv_sb.append(w)
        vbv_sb = persist.tile([128, 2], f32, name="vbv", tag="vbv")
        nc.sync.dma_start(vbv_sb[:], io["vbv"][:, :])
        for f in range(2):
            w = persist.tile([128, 512], dt_proj, name=f"pjt{f}", tag=f"pjt{f}")
            nc.scalar.dma_start(w[:], io["pjt"][f * 128:(f + 1) * 128, :])
            pjt_sb.append(w)

        # ---- depthwise convs (DVE, bf16 2x mode; inputs on the Pool queue)
        with tc.tile_pool(name="conv", bufs=2) as convp:
            for src, wv_, dst in (("mt", mcw_sb, QP), ("st", scw_sb, KP)):
                for t in range(2):
                    xin = convp.tile([128, N], dt_conv, name=f"ci_{src}{t}",
                                     tag="cin")
                    nc.gpsimd.dma_start(xin[:], io[src][t * 128:(t + 1) * 128, :])
                    y = convp.tile([128, N], dt_conv, name=f"cy_{src}{t}",
                                   tag="cy", bufs=1)
                    w0, w1, w2, cb = (wv_[:, 4 * t + k:4 * t + k + 1]
                                      for k in range(4))
                    nc.vector.tensor_scalar(y[:], xin[:], w1, cb,
                                            ALU.mult, ALU.add)
                    nc.vector.scalar_tensor_tensor(
                        y[:, 1:], xin[:, :N - 1], w0, y[:, 1:],
                        ALU.mult, ALU.add)
                    nc.vector.scalar_tensor_tensor(
                        y[:, :N - 1], xin[:, 1:], w2, y[:, :N - 1],
                        ALU.mult, ALU.add)
                    nc.vector.tensor_copy(dst[2 * t + 1][0:64, :], y[0:64, :])
                    nc.vector.tensor_copy(dst[2 * t][64:128, :], y[64:128, :])

        # ---- q/k projections, first half (chunks 0,1 = heads 0,1).
        # Bias-adds run on ACT (idle until the first exp); the PSUM pool
        # closes before attention so its banks are reusable there.
        def qk_chunk_matmuls(ps, ch, qb):
            for ih in range(NIH):
                hqs = slice(qb * FBS + ih * FB, qb * FBS + (ih + 1) * FB)
                for c in range(4):
                    nc.tensor.matmul(
                        ps[:, ih * FB:(ih + 1) * FB],
                        wqk_sb[c][:, ch * 128:(ch + 1) * 128],
                        xt_sb[c][:, hqs],
                        start=(c == 0), stop=(c == 3))

        def qk_bias_out(ps, ch, qb, engine):
            t = ch // 2
            qs = slice(qb * FBS, (qb + 1) * FBS)
            dst = QP if ch % 2 == 0 else KP
            if engine == "act":
                nc.scalar.activation(dst[2 * t][0:64, qs], ps[0:64, :],
                                     AF.Identity, bias=qkb_sb[0:64, ch:ch + 1])
                nc.scalar.activation(dst[2 * t + 1][64:128, qs], ps[64:128, :],
                                     AF.Identity,
                                     bias=qkb_sb[64:128, ch:ch + 1])
            else:
                nc.vector.tensor_scalar_add(dst[2 * t][0:64, qs], ps[0:64, :],
                                            qkb_sb[0:64, ch:ch + 1])
                nc.vector.tensor_scalar_add(dst[2 * t + 1][64:128, qs],
                                            ps[64:128, :],
                                            qkb_sb[64:128, ch:ch + 1])

        with tc.tile_pool(name="ps_qkv", bufs=1, space="PSUM") as ps_qkp:
            for ch in range(4):
                for qb in range(NQ2):
                    ps = ps_qkp.tile([128, FBS], f32, name="psqk", tag="psqk",
                                     bufs=3)
                    qk_chunk_matmuls(ps, ch, qb)
                    qk_bias_out(ps, ch, qb, "act")

        # ---- v projection (needed before the first head's attnV)
        with tc.tile_pool(name="ps_v", bufs=1, space="PSUM") as ps_vp:
            for blk in range(NKB):
                bs = slice(blk * 128, (blk + 1) * 128)
                ps = ps_vp.tile([128, 256], f32, name="psv", tag="psv",
                                bufs=2)
                for c in range(4):
                    nc.tensor.matmul(ps[:], xt_sb[c][:, bs], wv_sb[c][:],
                                     start=(c == 0), stop=(c == 3))
                v3 = v_sb[blk].rearrange("p (h f) -> p h f", h=HPC)
                nc.vector.tensor_copy(v3[:, :, 0:64],
                                      ps.rearrange("p (h f) -> p h f", h=HPC))
                # memset lacks float32r support; write the ones/pad columns
                # through an f32 view (identical bit pattern)
                ones_ap, pad_ap = v3[:, :, 64:65], v3[:, :, 65:66]
                if dt_attn == dt.float32r:
                    ones_ap = ones_ap.bitcast(f32)
                    pad_ap = pad_ap.bitcast(f32)
                nc.vector.memset(ones_ap, 1.0)
                nc.vector.memset(pad_ap, 0.0)

        # ---- attention + per-stripe normalize/project/output.
        # PSUM budget (8 banks): s_ps 2x2 + o_ps 1x2 + pj 2x1. The
        # projection of stripe 0 is deferred into stripe 1's first head
        # window so its matmuls don't stall the PE behind the normalize
        # dependency chain.
        with tc.tile_pool(name="ps_s", bufs=2, space="PSUM") as ps_sp, \
                tc.tile_pool(name="ps_o", bufs=1, space="PSUM") as ps_op, \
                tc.tile_pool(name="ps_pj", bufs=2, space="PSUM") as ps_pjp, \
                tc.tile_pool(name="esbp", bufs=2) as esbp, \
                tc.tile_pool(name="stg", bufs=2) as stgp, \
                tc.tile_pool(name="osbp", bufs=3) as osbp:

            def stripe_norm(q2):
                """Denominator reciprocal + broadcast + normalize (no PE)."""
                cs = slice(q2 * FBS, (q2 + 1) * FBS)
                # stripe q2's denominators live in rows q2*64 + h*16, so one
                # 32-aligned 64-partition reciprocal covers all four heads
                nc.vector.reciprocal(denR[q2 * 64:(q2 + 1) * 64, :],
                                     denR[q2 * 64:(q2 + 1) * 64, :])
                for t in range(2):
                    for par in range(2):
                        h = 2 * t + par
                        r0 = q2 * 64 + h * 16
                        nc.sync.dma_start(io["drec"][h:h + 1, cs],
                                          denR[r0:r0 + 16, :])
                        # casting broadcast goes through the Pool queue
                        nc.gpsimd.dma_start(
                            bcsb[t][par * 64:(par + 1) * 64, cs],
                            io["drec"][h:h + 1, cs].broadcast_to([64, FBS]))
                    nc.vector.tensor_mul(aT[t][:, cs], aT[t][:, cs],
                                         bcsb[t][:, cs])
                    nc.vector.tensor_scalar_add(aT[t][:, cs], aT[t][:, cs],
                                                vbv_sb[:, t:t + 1])

            def proj_blk(blk):
                bs = slice(blk * 128, (blk + 1) * 128)
                pj = ps_pjp.tile([128, FB], f32, name="pj", tag="pj")
                for f in range(2):
                    nc.tensor.matmul(pj[:], aT[f][:, bs], pjt_sb[f][:],
                                     start=(f == 0), stop=(f == 1))
                ob = osbp.tile([128, FB], f32, name="ob", tag="ob")
                nc.vector.tensor_copy(ob[:], pj[:])
                nc.gpsimd.dma_start(io["out"][bs, :], ob[:])

            def head_block(q2, h, fill=None):
                qbase = q2 * FBS
                cs = slice(qbase, qbase + FBS)
                t, odd = h // 2, h % 2
                vcols = slice(h * 66, (h + 1) * 66)
                o_ps = ps_op.tile([66, FBS], f32, name=f"ops{h}_{q2}",
                                  tag="ops")

                def attn_v(nk, e_sb):
                    for ih in range(NIH):
                        nc.tensor.matmul(
                            o_ps[:, ih * FB:(ih + 1) * FB],
                            v_sb[nk][:, vcols],
                            e_sb[:, ih * FB:(ih + 1) * FB],
                            start=(nk == 0), stop=(nk == NKB - 1))

                e_prev = None
                for nk in range(NKB):
                    if fill is not None:
                        fill(nk)
                    ks = slice(nk * 128, (nk + 1) * 128)
                    s_ps = ps_sp.tile([128, FBS], f32, name="sps", tag="sps")
                    for ih in range(NIH):
                        hqs = slice(qbase + ih * FB, qbase + (ih + 1) * FB)
                        nc.tensor.matmul(s_ps[:, ih * FB:(ih + 1) * FB],
                                         KP[h][:, ks], QP[h][:, hqs],
                                         start=True, stop=True)
                    e_sb = esbp.tile([128, FBS], dt_attn, name="esb",
                                     tag="esb")
                    nc.scalar.activation(e_sb[:], s_ps[:], AF.Exp)
                    if e_prev is not None:
                        attn_v(nk - 1, e_prev)
                    e_prev = e_sb
                attn_v(NKB - 1, e_prev)

                # lane-locked engines cannot shift partitions and DMA
                # cannot read PSUM, so shifting copies stage through SBUF
                stgd = stgp.tile([65, FBS], f32, name=f"sd{h}_{q2}",
                                 tag="stgd")
                if odd:
                    stg = stgp.tile([64, FBS], dt_proj, name=f"sg{h}_{q2}",
                                    tag="stg")
                    nc.vector.tensor_copy(stg[:], o_ps[0:64, :])
                    nc.sync.dma_start(aT[t][64:128, cs], stg[:])
                else:
                    nc.vector.tensor_copy(aT[t][0:64, cs], o_ps[0:64, :])
                nc.vector.tensor_copy(stgd[64:65, :], o_ps[64:65, :])
                # denominator stripe -> denR rows (DMA-reshaped, linear)
                r0 = q2 * 64 + h * 16
                nc.sync.dma_start(denR[r0:r0 + 16, :], stgd[64:65, :])

            for h in range(HPC):
                head_block(0, h)
            stripe_norm(0)
            # stripe 1, first head: emit stripe 0's projection one block
            # per nk once the normalize chain has had time to complete
            head_block(1, 0,
                       fill=lambda nk: proj_blk(nk - 6) if 6 <= nk < 14
                       else None)
            for h in range(1, HPC):
                head_block(1, h)
            stripe_norm(1)
            for blk in range(FBS // 128, N // 128):
                proj_blk(blk)


def _build(cfg_key):
    from concourse import bacc, mybir, tile

    cfg = dict(cfg_key)
    dt = mybir.dt
    nc = bacc.Bacc("TRN2", target_bir_lowering=False, debug=False,
                   num_devices=8)
    _d = {"f32": dt.float32, "f32r": dt.float32r, "bf16": dt.bfloat16}
    dt_qkv = _d[cfg["qkv"]]
    dt_proj = _d[cfg["proj"]]
    dt_conv = dt.bfloat16 if cfg["attn"] == "bf16" else dt.float32
    shapes = {
        "xt": ([DIM, N], dt_qkv), "mt": ([256, N], dt_conv),
        "st": ([256, N], dt_conv),
        "wqk": ([DIM, 512], dt_qkv), "wv": ([DIM, 256], dt_qkv),
        "pjt": ([256, DIM], dt_proj),
        "mcw": ([128, 8], dt.float32),
        "scw": ([128, 8], dt.float32),
        "qkb": ([128, 4], dt.float32), "vbv": ([128, 2], dt.float32),
    }
    io = {}
    for name, (shape, dtt) in shapes.items():
        io[name] = nc.dram_tensor(name, shape, dtt,
                                  kind="ExternalInput").ap()
    io["out"] = nc.dram_tensor("out", [N, DIM], dt.float32,
                               kind="ExternalOutput").ap()
    # internal DRAM bounce for the denominator broadcast (DMA cannot
    # replicate from an SBUF source, but a DRAM source AP is linear and
    # supports a zero-step leading dim)
    io["drec"] = nc.dram_tensor("drec", [4, N], dt.float32).ap()
    with tile.TileContext(nc) as tc:
        _emit(tc, nc, io, cfg)
    nc.compile()
    return nc


def _get_program(cfg):
    key = tuple(sorted(cfg.items()))
    if key not in _CACHE:
        _CACHE[key] = _build(key)
    return _CACHE[key]


# ------------------------------------------------------------------ wrapper
def kernel(_cfg=None, _want_results=False, **inputs):
    from concourse.bass_utils import run_bass_kernel_spmd

    cfg = dict(_DEFAULT_CFG)
    if _cfg:
        cfg.update(_cfg)
    env_cfg = os.environ.get("BASSKERN_CFG")
    if env_cfg:  # e.g. "attn=f32r,qkv=f32r"
        for kv in env_cfg.split(","):
            k, v = kv.split("=")
            cfg[k] = v

    inputs = {k: np.asarray(v, dtype=np.float32) for k, v in inputs.items()}
    nc = _get_program(cfg)
    in_maps = [_host_prep(core, inputs, cfg) for core in range(8)]
    res = run_bass_kernel_spmd(nc, in_maps, list(range(8)))

    out = np.empty((B, N, DIM), np.float32)
    pb = inputs["proj_b"]
    for b in range(B):
        out[b] = res.results[2 * b]["out"] + res.results[2 * b + 1]["out"] + pb
    if _want_results:
        return out, res
    return out


# revision 18
# speedup vs baseline: 1.3575x; 1.1730x over previous
"""Trainium2 Bass kernel for nn_AttentionWithVQ (B=4, N=2048, DIM=512, H=8,
depthwise-conv positional term, softmax attention, output projection).

Sharding: data-parallel over B (4 batches x 2 core-groups) and tensor-parallel
over heads (4 heads per core) -> 8 cores, fully independent per core except a
final partial-sum over the two head-groups of each batch, done on host at
gather time (the output projection contracts over heads).

Core algorithmic fusion: the score matrix
    S = 0.5*(scale * q @ k^T + scale * conv1(m) @ conv2(s)^T)
is ONE matmul over a concatenated 128-feature axis:
    S = Qp^T @ Kp,  Qp = [q*scale*0.5 ; conv1(m)*scale*0.5], Kp = [k ; conv2(s)]
which exactly fills the 128x128 PE array contraction dim.

Softmax denominators come for free by appending a ones-column to V
(attn@[V|1] yields the row-sums of exp(S) in the last output row); exp() is
numerically safe without max-subtraction for this problem's score magnitudes
(|S| < ~1 given the 0.02-scaled weights).

Partition alignment: compute engines are lane-locked (PSUM partition p ->
SBUF partition p), so per-head feature layouts alternate by head parity
(even heads [qk;conv], odd heads [conv;qk]) making every PSUM->SBUF copy
partition-aligned; the few genuinely shifting copies (odd-head attention
outputs, denominator rows) go through DMA, which can move partitions freely.
All permutation bookkeeping is done host-side in numpy when preparing
per-core inputs.

Schedule (v2): the kernel is a single software-pipelined stream ordered to
keep the PE and ACT engines saturated end-to-end:
  - input DMAs are split across the SP/ACT/Pool queues with the
    qkv-critical tensors (xt, wqk) first;
  - qkv bias-adds run on the ACT engine (idle until the first exp);
  - attention runs stripe-outer (q 1024-blocks) / head-inner, with
    scores(nk+1) emitted before attnV(nk) so exp latency is hidden, the
    v-projection matmuls interleaved into the first head's window and the
    second half of the q/k projection into the second head's window;
  - each stripe is normalized, projected, and DMA'd out as soon as its
    4 heads finish, overlapping the next stripe's attention.
"""

import os
import sys

sys.path.insert(0, "/opt/trn_rl_repo")

import numpy as np

# ---------------------------------------------------------------- constants
B, N, DIM, HEAD, VQE_K = 4, 2048, 512, 8, 3
Dh = DIM // HEAD            # 64
HPC = HEAD // 2             # heads per core (8 cores = 4 batch * 2 groups)
P = 128
FB = 512                    # one fp32 PSUM bank
NKB = N // P                # 16
SCALE_Q = Dh ** -0.5 * 0.5  # folds the 0.5 score scale into the q/conv1 side

_DEFAULT_CFG = {"qkv": "bf16", "attn": "bf16", "proj": "bf16"}

_CACHE = {}


def _np_dt(v):
    if v == "bf16":
        import ml_dtypes
        return ml_dtypes.bfloat16
    return np.float32


# ---------------------------------------------------------------- host prep
def _host_prep(core, inp, cfg=None):
    """Build the per-core input arrays (sharding + layout permutations)."""
    cfg = cfg or _DEFAULT_CFG
    b, g = core // 2, core % 2
    f32 = np.float32
    x, m, s = inp["x"], inp["m"], inp["s"]
    qkv_w, qkv_b = inp["qkv_w"], inp["qkv_b"]
    proj_w = inp["proj_w"]
    p1w = inp["pe1_w"].reshape(HEAD, VQE_K)
    p2w = inp["pe2_w"].reshape(HEAD, VQE_K)
    pe1_b, pe2_b = inp["pe1_b"], inp["pe2_b"]
    dt_qkv = _np_dt(cfg["qkv"])
    dt_proj = _np_dt(cfg["proj"])
    dt_conv = _np_dt("bf16" if cfg["attn"] == "bf16" else "f32")

    d = {}
    d["xt"] = np.ascontiguousarray(x[b].T).astype(dt_qkv)  # [512, 2048]

    # m/s transposed, tile t rows = [head(2t+1) feats ; head(2t) feats]
    mt = np.empty((256, N), f32)
    st = np.empty((256, N), f32)
    mcw = np.zeros((128, 8), f32)
    scw = np.zeros((128, 8), f32)
    for t in range(2):
        h_lo, h_hi = g * 4 + 2 * t + 1, g * 4 + 2 * t
        mt[t * 128:t * 128 + 64] = m[b][:, h_lo * 64:(h_lo + 1) * 64].T
        mt[t * 128 + 64:t * 128 + 128] = m[b][:, h_hi * 64:(h_hi + 1) * 64].T
        st[t * 128:t * 128 + 64] = s[b][:, h_lo * 64:(h_lo + 1) * 64].T
        st[t * 128 + 64:t * 128 + 128] = s[b][:, h_hi * 64:(h_hi + 1) * 64].T
        for p in range(128):
            h = g * 4 + 2 * t + (1 if p < 64 else 0)
            mcw[p, 4 * t:4 * t + 3] = p1w[h] * SCALE_Q
            scw[p, 4 * t:4 * t + 3] = p2w[h]
            mcw[p, 4 * t + 3] = pe1_b[h] * SCALE_Q
            scw[p, 4 * t + 3] = pe2_b[h]
    d["mt"], d["st"] = mt.astype(dt_conv), st.astype(dt_conv)
    d["mcw"], d["scw"] = mcw, scw

    # q/k projection weights: chunk ch=(t, q|k) = [even-head rows; odd-head rows]
    wqk_f = np.empty((512, DIM), f32)
    qkb = np.zeros((128, 4), f32)
    for t in range(2):
        for j in range(2):  # 0=q, 1=k
            ch = 2 * t + j
            h_e, h_o = g * 4 + 2 * t, g * 4 + 2 * t + 1
            base = j * DIM
            wqk_f[ch * 128:ch * 128 + 64] = qkv_w[base + h_e * 64:base + (h_e + 1) * 64]
            wqk_f[ch * 128 + 64:(ch + 1) * 128] = qkv_w[base + h_o * 64:base + (h_o + 1) * 64]
            qkb[0:64, ch] = qkv_b[base + h_e * 64:base + (h_e + 1) * 64]
            qkb[64:128, ch] = qkv_b[base + h_o * 64:base + (h_o + 1) * 64]
            if j == 0:
                wqk_f[ch * 128:(ch + 1) * 128] *= SCALE_Q
                qkb[:, ch] *= SCALE_Q
    d["wqk"] = np.ascontiguousarray(wqk_f.T).astype(dt_qkv)  # [c=512, f=512]
    d["qkb"] = qkb

    d["wv"] = np.ascontiguousarray(
        qkv_w[2 * DIM + g * 256:2 * DIM + (g + 1) * 256].T).astype(dt_qkv)

    # proj rows / v-bias in aT partition order:
    # aT tile t partition p -> head 2t+(p>=64), d=p%64
    pjt = np.empty((256, DIM), f32)
    vbv = np.empty((256,), f32)
    for t in range(2):
        for p in range(128):
            h_l = 2 * t + (1 if p >= 64 else 0)
            h = g * 4 + h_l
            pjt[t * 128 + p] = proj_w[:, h * 64 + (p % 64)]
            vbv[t * 128 + p] = qkv_b[2 * DIM + h * 64 + (p % 64)]
    d["pjt"] = pjt.astype(dt_proj)
    d["vbv"] = np.ascontiguousarray(vbv.reshape(2, 128).T.astype(f32))  # [128, 2]
    return d


# ------------------------------------------------------------- device build
def _emit(tc, nc, io, cfg):
    from contextlib import ExitStack

    from concourse import mybir

    dt = mybir.dt
    f32 = dt.float32
    AF = mybir.ActivationFunctionType
    ALU = mybir.AluOpType

    def _dt(v):
        return {"f32": f32, "f32r": dt.float32r, "bf16": dt.bfloat16}[v]

    dt_qkv = _dt(cfg["qkv"])
    dt_attn = _dt(cfg["attn"])
    dt_proj = _dt(cfg["proj"])
    dt_conv = dt.bfloat16 if dt_attn == dt.bfloat16 else f32
    # exp granularity: one ACT op per FBS-wide stripe (matmuls within are
    # still 512-wide: a matmul output cannot cross a PSUM bank)
    FBS = 1024
    NIH = FBS // FB           # 2
    NQ2 = N // FBS            # 2 q-stripes

    with ExitStack() as ctx:
        persist = ctx.enter_context(tc.tile_pool(name="persist", bufs=1))

        # ---- persistent tiles
        wqk_sb, wv_sb, xt_sb = [], [], []
        QP, KP, v_sb, aT, pjt_sb = [], [], [], [], []
        # critical-path inputs first: xt on the SP queue, wqk on the ACT
        # queue so both stream concurrently from t=0
        for c in range(4):
            xt = persist.tile([128, N], dt_qkv, name=f"xt{c}", tag=f"xt{c}")
            nc.sync.dma_start(xt[:], io["xt"][c * 128:(c + 1) * 128, :])
            xt_sb.append(xt)
        for c in range(4):
            w = persist.tile([128, 512], dt_qkv, name=f"wqk{c}", tag=f"wqk{c}")
            nc.scalar.dma_start(w[:], io["wqk"][c * 128:(c + 1) * 128, :])
            wqk_sb.append(w)
        mcw_sb = persist.tile([128, 8], f32, name="mcw", tag="mcw")
        nc.gpsimd.dma_start(mcw_sb[:], io["mcw"][:, :])
        scw_sb = persist.tile([128, 8], f32, name="scw", tag="scw")
        nc.gpsimd.dma_start(scw_sb[:], io["scw"][:, :])
        qkb_sb = persist.tile([128, 4], f32, name="qkb", tag="qkb")
        nc.sync.dma_start(qkb_sb[:], io["qkb"][:, :])

        for h in range(HPC):
            QP.append(persist.tile([128, N], dt_attn, name=f"QP{h}", tag=f"QP{h}"))
            KP.append(persist.tile([128, N], dt_attn, name=f"KP{h}", tag=f"KP{h}"))
        # per-head V block is [v(64) | ones | zero-pad] = 66 columns: matmul
        # operands need 4-byte-aligned offsets, so the block width must be
        # even for 2-byte dtypes (66*h*2 is always 4-aligned)
        for blk in range(NKB):
            v_sb.append(persist.tile([128, HPC * 66], dt_attn, name=f"vsb{blk}",
                                     tag=f"vsb{blk}"))
        for t in range(2):
            aT.append(persist.tile([128, N], dt_proj, name=f"aT{t}", tag=f"aT{t}"))
        # softmax denominators, DMA-reshaped onto 32-partition tiles so the
        # (slow per-element) reciprocal runs 32x wider than a row layout
        denR = persist.tile([128, N // 32], f32, name="denR", tag="denR")
        # stationary 0/1 patterns for the denominator-broadcast matmul:
        # lhsT = bcpat[64:65, par*128:(par+1)*128] replicates a [1, FB] den
        # row onto the head's 64-partition half of a PSUM tile (bf16: the
        # BIR verifier requires f32r operands to be produced as f32r, which
        # memset/copy cannot do)
        bcpat = persist.tile([128, 256], dt.bfloat16, name="bcpat",
                             tag="bcpat")
        nc.vector.memset(bcpat[:, 0:64], 1.0)
        nc.vector.memset(bcpat[:, 64:192], 0.0)
        nc.vector.memset(bcpat[:, 192:256], 1.0)

        for c in range(4):
            w = persist.tile([128, 256], dt_qkv, name=f"wv{c}", tag=f"wv{c}")
            nc.scalar.dma_start(w[:], io["wv"][c * 128:(c + 1) * 128, :])
            wv_sb.append(w)
        vbv_sb = persist.tile([128, 2], f32, name="vbv", tag="vbv")
        nc.sync.dma_start(vbv_sb[:], io["vbv"][:, :])
        for f in range(2):
            w = persist.tile([128, 512], dt_proj, name=f"pjt{f}", tag=f"pjt{f}")
            nc.scalar.dma_start(w[:], io["pjt"][f * 128:(f + 1) * 128, :])
            pjt_sb.append(w)

        # ---- depthwise convs (DVE, bf16 2x mode; inputs on the Pool queue)
        # t=0 convs first: heads 0/1 gate the first attention window
        with tc.tile_pool(name="conv", bufs=2) as convp:
            for t in range(2):
                for src, wv_, dst in (("mt", mcw_sb, QP), ("st", scw_sb, KP)):
                    xin = convp.tile([128, N], dt_conv, name=f"ci_{src}{t}",
                                     tag="cin")
                    nc.gpsimd.dma_start(xin[:], io[src][t * 128:(t + 1) * 128, :])
                    y = convp.tile([128, N], dt_conv, name=f"cy_{src}{t}",
                                   tag="cy", bufs=1)
                    w0, w1, w2, cb = (wv_[:, 4 * t + k:4 * t + k + 1]
                                      for k in range(4))
                    nc.vector.tensor_scalar(y[:], xin[:], w1, cb,
                                            ALU.mult, ALU.add)
                    nc.vector.scalar_tensor_tensor(
                        y[:, 1:], xin[:, :N - 1], w0, y[:, 1:],
                        ALU.mult, ALU.add)
                    nc.vector.scalar_tensor_tensor(
                        y[:, :N - 1], xin[:, 1:], w2, y[:, :N - 1],
                        ALU.mult, ALU.add)
                    nc.vector.tensor_copy(dst[2 * t + 1][0:64, :], y[0:64, :])
                    nc.vector.tensor_copy(dst[2 * t][64:128, :], y[64:128, :])

        # ---- q/k projections, first half (chunks 0,1 = heads 0,1).
        # Bias-adds run on ACT (idle until the first exp); the PSUM pool
        # closes before attention so its banks are reusable there.
        def qk_chunk_matmuls(ps, ch, qb):
            for ih in range(NIH):
                hqs = slice(qb * FBS + ih * FB, qb * FBS + (ih + 1) * FB)
                for c in range(4):
                    nc.tensor.matmul(
                        ps[:, ih * FB:(ih + 1) * FB],
                        wqk_sb[c][:, ch * 128:(ch + 1) * 128],
                        xt_sb[c][:, hqs],
                        start=(c == 0), stop=(c == 3))

        def qk_bias_out(ps, ch, qb, engine):
            t = ch // 2
            qs = slice(qb * FBS, (qb + 1) * FBS)
            dst = QP if ch % 2 == 0 else KP
            if engine == "act":
                nc.scalar.activation(dst[2 * t][0:64, qs], ps[0:64, :],
                                     AF.Identity, bias=qkb_sb[0:64, ch:ch + 1])
                nc.scalar.activation(dst[2 * t + 1][64:128, qs], ps[64:128, :],
                                     AF.Identity,
                                     bias=qkb_sb[64:128, ch:ch + 1])
            else:
                nc.vector.tensor_scalar_add(dst[2 * t][0:64, qs], ps[0:64, :],
                                            qkb_sb[0:64, ch:ch + 1])
                nc.vector.tensor_scalar_add(dst[2 * t + 1][64:128, qs],
                                            ps[64:128, :],
                                            qkb_sb[64:128, ch:ch + 1])

        with tc.tile_pool(name="ps_qkv", bufs=1, space="PSUM") as ps_qkp:
            for ch in range(4):
                for qb in range(NQ2):
                    ps = ps_qkp.tile([128, FBS], f32, name="psqk", tag="psqk",
                                     bufs=3)
                    qk_chunk_matmuls(ps, ch, qb)
                    qk_bias_out(ps, ch, qb, "act")

        # ---- v projection (needed before the first head's attnV)
        with tc.tile_pool(name="ps_v", bufs=1, space="PSUM") as ps_vp:
            for blk in range(NKB):
                bs = slice(blk * 128, (blk + 1) * 128)
                ps = ps_vp.tile([128, 256], f32, name="psv", tag="psv",
                                bufs=2)
                for c in range(4):
                    nc.tensor.matmul(ps[:], xt_sb[c][:, bs], wv_sb[c][:],
                                     start=(c == 0), stop=(c == 3))
                v3 = v_sb[blk].rearrange("p (h f) -> p h f", h=HPC)
                nc.vector.tensor_copy(v3[:, :, 0:64],
                                      ps.rearrange("p (h f) -> p h f", h=HPC))
                # memset lacks float32r support; write the ones/pad columns
                # through an f32 view (identical bit pattern)
                ones_ap, pad_ap = v3[:, :, 64:65], v3[:, :, 65:66]
                if dt_attn == dt.float32r:
                    ones_ap = ones_ap.bitcast(f32)
                    pad_ap = pad_ap.bitcast(f32)
                nc.vector.memset(ones_ap, 1.0)
                nc.vector.memset(pad_ap, 0.0)

        # ---- attention: one flat software-pipelined (q2, h, nk) stream.
        # attnV lags scores/exp by one step so the pipeline never drains at
        # head boundaries; head drains, denominator normalizes, and the
        # stripe projections are injected as fill steps inside later head
        # windows to keep the PE stream dense.
        # PSUM budget (8 banks): s_ps 2x2 + o_ps 1x2 + pj/bc 2x1.
        with tc.tile_pool(name="ps_s", bufs=2, space="PSUM") as ps_sp, \
                tc.tile_pool(name="ps_o", bufs=1, space="PSUM") as ps_op, \
                tc.tile_pool(name="ps_pj", bufs=2, space="PSUM") as ps_pjp, \
                tc.tile_pool(name="esbp", bufs=2) as esbp, \
                tc.tile_pool(name="stg", bufs=2) as stgp, \
                tc.tile_pool(name="osbp", bufs=3) as osbp:
            o_tiles, stgd_t = {}, {}

            def attn_v(q2, h, nk, e_sb):
                # lazy o_ps alloc: first write lands after the previous
                # head's last attnV is already emitted (bufs=1 slot reuse)
                if nk == 0:
                    o_tiles[(q2, h)] = ps_op.tile([66, FBS], f32,
                                                  name=f"ops{h}_{q2}",
                                                  tag="ops")
                o_ps = o_tiles[(q2, h)]
                vcols = slice(h * 66, (h + 1) * 66)
                for ih in range(NIH):
                    nc.tensor.matmul(
                        o_ps[:, ih * FB:(ih + 1) * FB],
                        v_sb[nk][:, vcols],
                        e_sb[:, ih * FB:(ih + 1) * FB],
                        start=(nk == 0), stop=(nk == NKB - 1))

            def drain_head(q2, h):
                """aT copies + reciprocal of the denominator row."""
                t, odd = h // 2, h % 2
                cs = slice(q2 * FBS, (q2 + 1) * FBS)
                o_ps = o_tiles[(q2, h)]
                # lane-locked engines cannot shift partitions and DMA
                # cannot read PSUM, so shifting copies stage through SBUF
                stgd = stgp.tile([65, FBS], f32, name=f"sd{h}_{q2}",
                                 tag="stgd")
                if odd:
                    stg = stgp.tile([64, FBS], dt_proj, name=f"sg{h}_{q2}",
                                    tag="stg")
                    nc.vector.tensor_copy(stg[:], o_ps[0:64, :])
                    nc.sync.dma_start(aT[t][64:128, cs], stg[:])
                else:
                    nc.vector.tensor_copy(aT[t][0:64, cs], o_ps[0:64, :])
                nc.vector.tensor_copy(stgd[64:65, :], o_ps[64:65, :])
                # den row -> denR rows q2*64+h*16 (DMA-reshaped [16, 64])
                r0 = q2 * 64 + h * 16
                nc.sync.dma_start(denR[r0:r0 + 16, :], stgd[64:65, :])
                sdb = stgp.tile([65, FBS], dt.bfloat16, name=f"sb{h}_{q2}",
                                tag="sdb")
                stgd_t[(q2, h)] = sdb
                if odd:
                    # both heads of the t-group staged: one 32-aligned
                    # reciprocal, DMA-reshape back to rows, cast to bf16
                    nc.vector.reciprocal(denR[r0 - 16:r0 + 16, :],
                                         denR[r0 - 16:r0 + 16, :])
                    for hh in (h - 1, h):
                        rr = q2 * 64 + hh * 16
                        sdf = stgp.tile([65, FBS], f32, name=f"sf{hh}_{q2}",
                                        tag="sdf")
                        nc.sync.dma_start(sdf[64:65, :], denR[rr:rr + 16, :])
                        nc.vector.tensor_copy(stgd_t[(q2, hh)][64:65, :],
                                              sdf[64:65, :])

            def norm_half(q2, t, half):
                """PE-broadcast the 1/den rows of heads 2t,2t+1 and scale."""
                cols = slice(q2 * FBS + half * FB, q2 * FBS + (half + 1) * FB)
                bc = ps_pjp.tile([128, FB], f32, name="bc", tag="pj")
                for par in range(2):
                    sd = stgd_t[(q2, 2 * t + par)]
                    nc.tensor.matmul(
                        bc[:],
                        bcpat[64:65, par * 128:(par + 1) * 128],
                        sd[64:65, half * FB:(half + 1) * FB],
                        start=(par == 0), stop=(par == 1))
                nc.vector.tensor_mul(aT[t][:, cols], aT[t][:, cols], bc[:])
                nc.vector.tensor_scalar_add(aT[t][:, cols], aT[t][:, cols],
                                            vbv_sb[:, t:t + 1])

            def proj_blk(blk):
                bs = slice(blk * 128, (blk + 1) * 128)
                pj = ps_pjp.tile([128, FB], f32, name="pj", tag="pj")
                for f in range(2):
                    nc.tensor.matmul(pj[:], aT[f][:, bs], pjt_sb[f][:],
                                     start=(f == 0), stop=(f == 1))
                ob = osbp.tile([128, FB], f32, name="ob", tag="ob")
                nc.vector.tensor_copy(ob[:], pj[:])
                nc.gpsimd.dma_start(io["out"][bs, :], ob[:])

            # fill work injected at (q2, h, nk) steps: normalizes as soon as
            # both heads of a t-group drained, stripe-0 projection inside
            # stripe 1's first head window
            fills = {
                (0, 2, 2): [lambda: norm_half(0, 0, 0)],
                (0, 2, 3): [lambda: norm_half(0, 0, 1)],
                (1, 0, 2): [lambda: norm_half(0, 1, 0)],
                (1, 0, 3): [lambda: norm_half(0, 1, 1)],
                (1, 2, 2): [lambda: norm_half(1, 0, 0)],
                (1, 2, 3): [lambda: norm_half(1, 0, 1)],
            }
            for i in range(FBS // 128):
                fills.setdefault((1, 0, 6 + i), []).append(
                    lambda blk=i: proj_blk(blk))

            pend = None  # (q2, h, nk, e_sb)
            for q2 in range(NQ2):
                qbase = q2 * FBS
                for h in range(HPC):
                    for nk in range(NKB):
                        for f in fills.get((q2, h, nk), ()):
                            f()
                        ks = slice(nk * 128, (nk + 1) * 128)
                        s_ps = ps_sp.tile([128, FBS], f32, name="sps",
                                          tag="sps")
                        for ih in range(NIH):
                            hqs = slice(qbase + ih * FB, qbase + (ih + 1) * FB)
                            nc.tensor.matmul(s_ps[:, ih * FB:(ih + 1) * FB],
                                             KP[h][:, ks], QP[h][:, hqs],
                                             start=True, stop=True)
                        e_sb = esbp.tile([128, FBS], dt_attn, name="esb",
                                         tag="esb")
                        nc.scalar.activation(e_sb[:], s_ps[:], AF.Exp)
                        if pend is not None:
                            attn_v(*pend)
                            if pend[2] == NKB - 1:
                                drain_head(pend[0], pend[1])
                        pend = (q2, h, nk, e_sb)
            attn_v(*pend)
            drain_head(1, HPC - 1)
            norm_half(1, 1, 0)
            norm_half(1, 1, 1)
            for blk in range(FBS // 128, N // 128):
                proj_blk(blk)


def _build(cfg_key):
    from concourse import bacc, mybir, tile

    cfg = dict(cfg_key)
    dt = mybir.dt
    nc = bacc.Bacc("TRN2", target_bir_lowering=False, debug=False,
                   num_devices=8)
    _d = {"f32": dt.float32, "f32r": dt.float32r, "bf16": dt.bfloat16}
    dt_qkv = _d[cfg["qkv"]]
    dt_proj = _d[cfg["proj"]]
    dt_conv = dt.bfloat16 if cfg["attn"] == "bf16" else dt.float32
    shapes = {
        "xt": ([DIM, N], dt_qkv), "mt": ([256, N], dt_conv),
        "st": ([256, N], dt_conv),
        "wqk": ([DIM, 512], dt_qkv), "wv": ([DIM, 256], dt_qkv),
        "pjt": ([256, DIM], dt_proj),
        "mcw": ([128, 8], dt.float32),
        "scw": ([128, 8], dt.float32),
        "qkb": ([128, 4], dt.float32), "vbv": ([128, 2], dt.float32),
    }
    io = {}
    for name, (shape, dtt) in shapes.items():
        io[name] = nc.dram_tensor(name, shape, dtt,
                                  kind="ExternalInput").ap()
    io["out"] = nc.dram_tensor("out", [N, DIM], dt.float32,
                               kind="ExternalOutput").ap()
    with tile.TileContext(nc) as tc:
        _emit(tc, nc, io, cfg)
    nc.compile()
    return nc


def _get_program(cfg):
    key = tuple(sorted(cfg.items()))
    if key not in _CACHE:
        _CACHE[key] = _build(key)
    return _CACHE[key]


# ------------------------------------------------------------------ wrapper
def kernel(_cfg=None, _want_results=False, **inputs):
    from concourse.bass_utils import run_bass_kernel_spmd

    cfg = dict(_DEFAULT_CFG)
    if _cfg:
        cfg.update(_cfg)
    env_cfg = os.environ.get("BASSKERN_CFG")
    if env_cfg:  # e.g. "attn=f32r,qkv=f32r"
        for kv in env_cfg.split(","):
            k, v = kv.split("=")
            cfg[k] = v

    inputs = {k: np.asarray(v, dtype=np.float32) for k, v in inputs.items()}
    nc = _get_program(cfg)
    in_maps = [_host_prep(core, inputs, cfg) for core in range(8)]
    res = run_bass_kernel_spmd(nc, in_maps, list(range(8)))

    out = np.empty((B, N, DIM), np.float32)
    pb = inputs["proj_b"]
    for b in range(B):
        out[b] = res.results[2 * b]["out"] + res.results[2 * b + 1]["out"] + pb
    if _want_results:
        return out, res
    return out


# revision 23
# speedup vs baseline: 1.4034x; 1.0338x over previous
"""Trainium2 Bass kernel for nn_AttentionWithVQ (B=4, N=2048, DIM=512, H=8,
depthwise-conv positional term, softmax attention, output projection).

Sharding: data-parallel over B (4 batches x 2 core-groups) and tensor-parallel
over heads (4 heads per core) -> 8 cores, fully independent per core except a
final partial-sum over the two head-groups of each batch, done on host at
gather time (the output projection contracts over heads).

Core algorithmic fusion: the score matrix
    S = 0.5*(scale * q @ k^T + scale * conv1(m) @ conv2(s)^T)
is ONE matmul over a concatenated 128-feature axis:
    S = Qp^T @ Kp,  Qp = [q*scale*0.5 ; conv1(m)*scale*0.5], Kp = [k ; conv2(s)]
which exactly fills the 128x128 PE array contraction dim.

Softmax denominators come for free by appending a ones-column to V
(attn@[V|1] yields the row-sums of exp(S) in the last output row); exp() is
numerically safe without max-subtraction for this problem's score magnitudes
(|S| < ~1 given the 0.02-scaled weights).

Partition alignment: compute engines are lane-locked (PSUM partition p ->
SBUF partition p), so per-head feature layouts alternate by head parity
(even heads [qk;conv], odd heads [conv;qk]) making every PSUM->SBUF copy
partition-aligned; the few genuinely shifting copies (odd-head attention
outputs, denominator rows) go through DMA, which can move partitions freely.
All permutation bookkeeping is done host-side in numpy when preparing
per-core inputs.

Schedule (v2): the kernel is a single software-pipelined stream ordered to
keep the PE and ACT engines saturated end-to-end:
  - input DMAs are split across the SP/ACT/Pool queues with the
    qkv-critical tensors (xt, wqk) first;
  - qkv bias-adds run on the ACT engine (idle until the first exp);
  - attention runs stripe-outer (q 1024-blocks) / head-inner, with
    scores(nk+1) emitted before attnV(nk) so exp latency is hidden, the
    v-projection matmuls interleaved into the first head's window and the
    second half of the q/k projection into the second head's window;
  - each stripe is normalized, projected, and DMA'd out as soon as its
    4 heads finish, overlapping the next stripe's attention.
"""

import os
import sys

sys.path.insert(0, "/opt/trn_rl_repo")

import numpy as np

# ---------------------------------------------------------------- constants
B, N, DIM, HEAD, VQE_K = 4, 2048, 512, 8, 3
Dh = DIM // HEAD            # 64
HPC = HEAD // 2             # heads per core (8 cores = 4 batch * 2 groups)
P = 128
FB = 512                    # one fp32 PSUM bank
NKB = N // P                # 16
SCALE_Q = Dh ** -0.5 * 0.5  # folds the 0.5 score scale into the q/conv1 side

_DEFAULT_CFG = {"qkv": "bf16", "attn": "bf16", "proj": "bf16"}

_CACHE = {}


def _np_dt(v):
    if v == "bf16":
        import ml_dtypes
        return ml_dtypes.bfloat16
    return np.float32


# ---------------------------------------------------------------- host prep
def _host_prep(core, inp, cfg=None):
    """Build the per-core input arrays (sharding + layout permutations)."""
    cfg = cfg or _DEFAULT_CFG
    b, g = core // 2, core % 2
    f32 = np.float32
    x, m, s = inp["x"], inp["m"], inp["s"]
    qkv_w, qkv_b = inp["qkv_w"], inp["qkv_b"]
    proj_w = inp["proj_w"]
    p1w = inp["pe1_w"].reshape(HEAD, VQE_K)
    p2w = inp["pe2_w"].reshape(HEAD, VQE_K)
    pe1_b, pe2_b = inp["pe1_b"], inp["pe2_b"]
    dt_qkv = _np_dt(cfg["qkv"])
    dt_proj = _np_dt(cfg["proj"])
    dt_conv = _np_dt("bf16" if cfg["attn"] == "bf16" else "f32")

    d = {}
    d["xt"] = np.ascontiguousarray(x[b].T).astype(dt_qkv)  # [512, 2048]

    # m/s transposed, tile t rows = [head(2t+1) feats ; head(2t) feats]
    mt = np.empty((256, N), f32)
    st = np.empty((256, N), f32)
    mcw = np.zeros((128, 8), f32)
    scw = np.zeros((128, 8), f32)
    for t in range(2):
        h_lo, h_hi = g * 4 + 2 * t + 1, g * 4 + 2 * t
        mt[t * 128:t * 128 + 64] = m[b][:, h_lo * 64:(h_lo + 1) * 64].T
        mt[t * 128 + 64:t * 128 + 128] = m[b][:, h_hi * 64:(h_hi + 1) * 64].T
        st[t * 128:t * 128 + 64] = s[b][:, h_lo * 64:(h_lo + 1) * 64].T
        st[t * 128 + 64:t * 128 + 128] = s[b][:, h_hi * 64:(h_hi + 1) * 64].T
        for p in range(128):
            h = g * 4 + 2 * t + (1 if p < 64 else 0)
            mcw[p, 4 * t:4 * t + 3] = p1w[h] * SCALE_Q
            scw[p, 4 * t:4 * t + 3] = p2w[h]
            mcw[p, 4 * t + 3] = pe1_b[h] * SCALE_Q
            scw[p, 4 * t + 3] = pe2_b[h]
    d["mt"], d["st"] = mt.astype(dt_conv), st.astype(dt_conv)
    d["mcw"], d["scw"] = mcw, scw

    # q/k projection weights: chunk ch=(t, q|k) = [even-head rows; odd-head rows]
    wqk_f = np.empty((512, DIM), f32)
    qkb = np.zeros((128, 4), f32)
    for t in range(2):
        for j in range(2):  # 0=q, 1=k
            ch = 2 * t + j
            h_e, h_o = g * 4 + 2 * t, g * 4 + 2 * t + 1
            base = j * DIM
            wqk_f[ch * 128:ch * 128 + 64] = qkv_w[base + h_e * 64:base + (h_e + 1) * 64]
            wqk_f[ch * 128 + 64:(ch + 1) * 128] = qkv_w[base + h_o * 64:base + (h_o + 1) * 64]
            qkb[0:64, ch] = qkv_b[base + h_e * 64:base + (h_e + 1) * 64]
            qkb[64:128, ch] = qkv_b[base + h_o * 64:base + (h_o + 1) * 64]
            if j == 0:
                wqk_f[ch * 128:(ch + 1) * 128] *= SCALE_Q
                qkb[:, ch] *= SCALE_Q
    d["wqk"] = np.ascontiguousarray(wqk_f.T).astype(dt_qkv)  # [c=512, f=512]
    d["qkb"] = qkb

    d["wv"] = np.ascontiguousarray(
        qkv_w[2 * DIM + g * 256:2 * DIM + (g + 1) * 256].T).astype(dt_qkv)

    # proj rows / v-bias in aT partition order:
    # aT tile t partition p -> head 2t+(p>=64), d=p%64
    pjt = np.empty((256, DIM), f32)
    vbv = np.empty((256,), f32)
    for t in range(2):
        for p in range(128):
            h_l = 2 * t + (1 if p >= 64 else 0)
            h = g * 4 + h_l
            pjt[t * 128 + p] = proj_w[:, h * 64 + (p % 64)]
            vbv[t * 128 + p] = qkv_b[2 * DIM + h * 64 + (p % 64)]
    d["pjt"] = pjt.astype(dt_proj)
    d["vbv"] = np.ascontiguousarray(vbv.reshape(2, 128).T.astype(f32))  # [128, 2]
    return d


# ------------------------------------------------------------- device build
def _emit(tc, nc, io, cfg):
    from contextlib import ExitStack

    from concourse import mybir

    dt = mybir.dt
    f32 = dt.float32
    AF = mybir.ActivationFunctionType
    ALU = mybir.AluOpType

    def _dt(v):
        return {"f32": f32, "f32r": dt.float32r, "bf16": dt.bfloat16}[v]

    dt_qkv = _dt(cfg["qkv"])
    dt_attn = _dt(cfg["attn"])
    dt_proj = _dt(cfg["proj"])
    dt_conv = dt.bfloat16 if dt_attn == dt.bfloat16 else f32
    # exp granularity: one ACT op per FBS-wide stripe (matmuls within are
    # still 512-wide: a matmul output cannot cross a PSUM bank)
    FBS = 1024
    NIH = FBS // FB           # 2
    NQ2 = N // FBS            # 2 q-stripes

    with ExitStack() as ctx:
        persist = ctx.enter_context(tc.tile_pool(name="persist", bufs=1))

        # ---- persistent tiles
        wqk_sb, wv_sb, xt_sb = [], [], []
        QP, KP, v_sb, aT, pjt_sb = [], [], [], [], []
        # critical-path inputs first: xt on the SP queue, wqk on the ACT
        # queue so both stream concurrently from t=0
        for c in range(4):
            xt = persist.tile([128, N], dt_qkv, name=f"xt{c}", tag=f"xt{c}")
            nc.sync.dma_start(xt[:], io["xt"][c * 128:(c + 1) * 128, :])
            xt_sb.append(xt)
        for c in range(4):
            w = persist.tile([128, 512], dt_qkv, name=f"wqk{c}", tag=f"wqk{c}")
            nc.scalar.dma_start(w[:], io["wqk"][c * 128:(c + 1) * 128, :])
            wqk_sb.append(w)
        mcw_sb = persist.tile([128, 8], f32, name="mcw", tag="mcw")
        nc.gpsimd.dma_start(mcw_sb[:], io["mcw"][:, :])
        scw_sb = persist.tile([128, 8], f32, name="scw", tag="scw")
        nc.gpsimd.dma_start(scw_sb[:], io["scw"][:, :])
        qkb_sb = persist.tile([128, 4], f32, name="qkb", tag="qkb")
        nc.sync.dma_start(qkb_sb[:], io["qkb"][:, :])

        for h in range(HPC):
            QP.append(persist.tile([128, N], dt_attn, name=f"QP{h}", tag=f"QP{h}"))
            KP.append(persist.tile([128, N], dt_attn, name=f"KP{h}", tag=f"KP{h}"))
        # per-head V block is [v(64) | ones | zero-pad] = 66 columns: matmul
        # operands need 4-byte-aligned offsets, so the block width must be
        # even for 2-byte dtypes (66*h*2 is always 4-aligned)
        for blk in range(NKB):
            v_sb.append(persist.tile([128, HPC * 66], dt_attn, name=f"vsb{blk}",
                                     tag=f"vsb{blk}"))
        for t in range(2):
            aT.append(persist.tile([128, N], dt_proj, name=f"aT{t}", tag=f"aT{t}"))
        # softmax denominators, DMA-reshaped onto 16-partition tiles so the
        # (slow per-element) reciprocal runs 16x wider than a row layout;
        # head h at rows h*32 (32-aligned for the DVE), stripe in columns
        denR = persist.tile([128, 128], f32, name="denR", tag="denR")
        # stationary 0/1 patterns for the denominator-broadcast matmul:
        # lhsT = bcpat[64:65, par*128:(par+1)*128] replicates a [1, FB] den
        # row onto the head's 64-partition half of a PSUM tile (bf16: the
        # BIR verifier requires f32r operands to be produced as f32r, which
        # memset/copy cannot do)
        bcpat = persist.tile([128, 256], dt.bfloat16, name="bcpat",
                             tag="bcpat")
        nc.vector.memset(bcpat[:, 0:64], 1.0)
        nc.vector.memset(bcpat[:, 64:192], 0.0)
        nc.vector.memset(bcpat[:, 192:256], 1.0)

        for c in range(4):
            w = persist.tile([128, 256], dt_qkv, name=f"wv{c}", tag=f"wv{c}")
            nc.scalar.dma_start(w[:], io["wv"][c * 128:(c + 1) * 128, :])
            wv_sb.append(w)
        vbv_sb = persist.tile([128, 2], f32, name="vbv", tag="vbv")
        nc.sync.dma_start(vbv_sb[:], io["vbv"][:, :])
        for f in range(2):
            w = persist.tile([128, 512], dt_proj, name=f"pjt{f}", tag=f"pjt{f}")
            nc.scalar.dma_start(w[:], io["pjt"][f * 128:(f + 1) * 128, :])
            pjt_sb.append(w)

        # ---- depthwise convs (DVE, bf16 2x mode; inputs on the Pool queue)
        # PE p-state warm-up: ~10 back-to-back dummy matmuls on the (tiny,
        # already-memset) bcpat tile during the input-DMA wait, so the real
        # qkv matmuls start at full clock instead of ramping through them
        with tc.tile_pool(name="ps_warm", bufs=1, space="PSUM") as ps_wp:
            wps = ps_wp.tile([128, 256], f32, name="warm", tag="warm")
            for _ in range(10):
                nc.tensor.matmul(wps[:], bcpat[:, 0:128], bcpat[:, 0:256],
                                 start=True, stop=True)

        # t=0 convs first: heads 0/1 gate the first attention window
        with tc.tile_pool(name="conv", bufs=2) as convp:
            for t in range(2):
                for src, wv_, dst in (("mt", mcw_sb, QP), ("st", scw_sb, KP)):
                    xin = convp.tile([128, N], dt_conv, name=f"ci_{src}{t}",
                                     tag="cin")
                    nc.gpsimd.dma_start(xin[:], io[src][t * 128:(t + 1) * 128, :])
                    y = convp.tile([128, N], dt_conv, name=f"cy_{src}{t}",
                                   tag="cy", bufs=1)
                    w0, w1, w2, cb = (wv_[:, 4 * t + k:4 * t + k + 1]
                                      for k in range(4))
                    nc.vector.tensor_scalar(y[:], xin[:], w1, cb,
                                            ALU.mult, ALU.add)
                    nc.vector.scalar_tensor_tensor(
                        y[:, 1:], xin[:, :N - 1], w0, y[:, 1:],
                        ALU.mult, ALU.add)
                    nc.vector.scalar_tensor_tensor(
                        y[:, :N - 1], xin[:, 1:], w2, y[:, :N - 1],
                        ALU.mult, ALU.add)
                    nc.vector.tensor_copy(dst[2 * t + 1][0:64, :], y[0:64, :])
                    nc.vector.tensor_copy(dst[2 * t][64:128, :], y[64:128, :])

        # ---- q/k projections, first half (chunks 0,1 = heads 0,1).
        # Bias-adds run on ACT (idle until the first exp); the PSUM pool
        # closes before attention so its banks are reusable there.
        def qk_chunk_matmuls(ps, ch, qb):
            for ih in range(NIH):
                hqs = slice(qb * FBS + ih * FB, qb * FBS + (ih + 1) * FB)
                for c in range(4):
                    nc.tensor.matmul(
                        ps[:, ih * FB:(ih + 1) * FB],
                        wqk_sb[c][:, ch * 128:(ch + 1) * 128],
                        xt_sb[c][:, hqs],
                        start=(c == 0), stop=(c == 3))

        def qk_bias_out(ps, ch, qb, engine):
            t = ch // 2
            qs = slice(qb * FBS, (qb + 1) * FBS)
            dst = QP if ch % 2 == 0 else KP
            if engine == "act":
                nc.scalar.activation(dst[2 * t][0:64, qs], ps[0:64, :],
                                     AF.Identity, bias=qkb_sb[0:64, ch:ch + 1])
                nc.scalar.activation(dst[2 * t + 1][64:128, qs], ps[64:128, :],
                                     AF.Identity,
                                     bias=qkb_sb[64:128, ch:ch + 1])
            else:
                nc.vector.tensor_scalar_add(dst[2 * t][0:64, qs], ps[0:64, :],
                                            qkb_sb[0:64, ch:ch + 1])
                nc.vector.tensor_scalar_add(dst[2 * t + 1][64:128, qs],
                                            ps[64:128, :],
                                            qkb_sb[64:128, ch:ch + 1])

        with tc.tile_pool(name="ps_qkv", bufs=1, space="PSUM") as ps_qkp:
            for ch in range(4):
                for qb in range(NQ2):
                    ps = ps_qkp.tile([128, FBS], f32, name="psqk", tag="psqk",
                                     bufs=3)
                    qk_chunk_matmuls(ps, ch, qb)
                    qk_bias_out(ps, ch, qb, "act")

        # ---- v projection (needed before the first head's attnV)
        with tc.tile_pool(name="ps_v", bufs=1, space="PSUM") as ps_vp:
            for blk in range(NKB):
                bs = slice(blk * 128, (blk + 1) * 128)
                ps = ps_vp.tile([128, 256], f32, name="psv", tag="psv",
                                bufs=2)
                for c in range(4):
                    nc.tensor.matmul(ps[:], xt_sb[c][:, bs], wv_sb[c][:],
                                     start=(c == 0), stop=(c == 3))
                v3 = v_sb[blk].rearrange("p (h f) -> p h f", h=HPC)
                nc.vector.tensor_copy(v3[:, :, 0:64],
                                      ps.rearrange("p (h f) -> p h f", h=HPC))
                # memset lacks float32r support; write the ones/pad columns
                # through an f32 view (identical bit pattern)
                ones_ap, pad_ap = v3[:, :, 64:65], v3[:, :, 65:66]
                if dt_attn == dt.float32r:
                    ones_ap = ones_ap.bitcast(f32)
                    pad_ap = pad_ap.bitcast(f32)
                nc.vector.memset(ones_ap, 1.0)
                nc.vector.memset(pad_ap, 0.0)

        # ---- attention: one flat software-pipelined (q2, h, nk) stream.
        # attnV lags scores/exp by one step so the pipeline never drains at
        # head boundaries; head drains, denominator normalizes, and the
        # stripe projections are injected as fill steps inside later head
        # windows to keep the PE stream dense.
        # PSUM budget (8 banks): s_ps 2x2 + o_ps 1x2 + pj/bc 2x1.
        with tc.tile_pool(name="ps_s", bufs=2, space="PSUM") as ps_sp, \
                tc.tile_pool(name="ps_o", bufs=1, space="PSUM") as ps_op, \
                tc.tile_pool(name="ps_pj", bufs=2, space="PSUM") as ps_pjp, \
                tc.tile_pool(name="esbp", bufs=4) as esbp, \
                tc.tile_pool(name="stg", bufs=2) as stgp, \
                tc.tile_pool(name="osbp", bufs=3) as osbp:
            o_tiles, stgd_t = {}, {}

            def attn_v(q2, h, nk, e_sb):
                # lazy o_ps alloc: first write lands after the previous
                # head's last attnV is already emitted (bufs=1 slot reuse)
                if nk == 0:
                    o_tiles[(q2, h)] = ps_op.tile([66, FBS], f32,
                                                  name=f"ops{h}_{q2}",
                                                  tag="ops")
                o_ps = o_tiles[(q2, h)]
                vcols = slice(h * 66, (h + 1) * 66)
                for ih in range(NIH):
                    nc.tensor.matmul(
                        o_ps[:, ih * FB:(ih + 1) * FB],
                        v_sb[nk][:, vcols],
                        e_sb[:, ih * FB:(ih + 1) * FB],
                        start=(nk == 0), stop=(nk == NKB - 1))

            def drain_head(q2, h):
                """aT copies + reciprocal of the denominator row."""
                t, odd = h // 2, h % 2
                cs = slice(q2 * FBS, (q2 + 1) * FBS)
                o_ps = o_tiles[(q2, h)]
                # lane-locked engines cannot shift partitions and DMA
                # cannot read PSUM, so shifting copies stage through SBUF
                stgd = stgp.tile([65, FBS], f32, name=f"sd{h}_{q2}",
                                 tag="stgd")
                if odd:
                    stg = stgp.tile([64, FBS], dt_proj, name=f"sg{h}_{q2}",
                                    tag="stg")
                    nc.vector.tensor_copy(stg[:], o_ps[0:64, :])
                    nc.sync.dma_start(aT[t][64:128, cs], stg[:])
                else:
                    nc.vector.tensor_copy(aT[t][0:64, cs], o_ps[0:64, :])
                nc.vector.tensor_copy(stgd[64:65, :], o_ps[64:65, :])
                # den row -> denR [16, 64] block (rows h*32, cols q2*64):
                # reciprocal, DMA-reshape back to a row, cast to bf16
                dblk = denR[h * 32:h * 32 + 16, q2 * 64:(q2 + 1) * 64]
                nc.sync.dma_start(dblk, stgd[64:65, :])
                nc.vector.reciprocal(dblk, dblk)
                sdf = stgp.tile([65, FBS], f32, name=f"sf{h}_{q2}",
                                tag="sdf")
                nc.sync.dma_start(sdf[64:65, :], dblk)
                sdb = stgp.tile([65, FBS], dt.bfloat16, name=f"sb{h}_{q2}",
                                tag="sdb")
                nc.vector.tensor_copy(sdb[64:65, :], sdf[64:65, :])
                stgd_t[(q2, h)] = sdb

            def norm_half(q2, t, half):
                """PE-broadcast the 1/den rows of heads 2t,2t+1 and scale."""
                cols = slice(q2 * FBS + half * FB, q2 * FBS + (half + 1) * FB)
                bc = ps_pjp.tile([128, FB], f32, name="bc", tag="pj")
                for par in range(2):
                    sd = stgd_t[(q2, 2 * t + par)]
                    nc.tensor.matmul(
                        bc[:],
                        bcpat[64:65, par * 128:(par + 1) * 128],
                        sd[64:65, half * FB:(half + 1) * FB],
                        start=(par == 0), stop=(par == 1))
                nc.vector.tensor_mul(aT[t][:, cols], aT[t][:, cols], bc[:])
                nc.vector.tensor_scalar_add(aT[t][:, cols], aT[t][:, cols],
                                            vbv_sb[:, t:t + 1])

            def proj_blk(blk):
                bs = slice(blk * 128, (blk + 1) * 128)
                pj = ps_pjp.tile([128, FB], f32, name="pj", tag="pj")
                for f in range(2):
                    nc.tensor.matmul(pj[:], aT[f][:, bs], pjt_sb[f][:],
                                     start=(f == 0), stop=(f == 1))
                ob = osbp.tile([128, FB], f32, name="ob", tag="ob")
                nc.vector.tensor_copy(ob[:], pj[:])
                nc.gpsimd.dma_start(io["out"][bs, :], ob[:])

            # fill work injected at (q2, h, nk) steps: normalizes as soon as
            # both heads of a t-group drained, stripe-0 projection inside
            # stripe 1's first head window
            fills = {
                (0, 2, 2): [lambda: norm_half(0, 0, 0)],
                (0, 2, 3): [lambda: norm_half(0, 0, 1)],
                (1, 0, 2): [lambda: norm_half(0, 1, 0)],
                (1, 0, 3): [lambda: norm_half(0, 1, 1)],
                (1, 2, 2): [lambda: norm_half(1, 0, 0)],
                (1, 2, 3): [lambda: norm_half(1, 0, 1)],
            }
            for i in range(FBS // 128):
                fills.setdefault((1, 0, 6 + i), []).append(
                    lambda blk=i: proj_blk(blk))

            # attnV lags scores/exp via a queue: lag 3 across a head start
            # (so the previous head's o_ps drain — bufs=1 — never stalls the
            # PE), catching back up to lag 1 within the head
            pend_q = []

            def pop_pend():
                q2p, hp, nkp, e = pend_q.pop(0)
                attn_v(q2p, hp, nkp, e)
                if nkp == NKB - 1:
                    drain_head(q2p, hp)

            for q2 in range(NQ2):
                qbase = q2 * FBS
                for h in range(HPC):
                    for nk in range(NKB):
                        for f in fills.get((q2, h, nk), ()):
                            f()
                        ks = slice(nk * 128, (nk + 1) * 128)
                        s_ps = ps_sp.tile([128, FBS], f32, name="sps",
                                          tag="sps")
                        for ih in range(NIH):
                            hqs = slice(qbase + ih * FB, qbase + (ih + 1) * FB)
                            nc.tensor.matmul(s_ps[:, ih * FB:(ih + 1) * FB],
                                             KP[h][:, ks], QP[h][:, hqs],
                                             start=True, stop=True)
                        e_sb = esbp.tile([128, FBS], dt_attn, name="esb",
                                         tag="esb")
                        nc.scalar.activation(e_sb[:], s_ps[:], AF.Exp)
                        pend_q.append((q2, h, nk, e_sb))
                        while len(pend_q) > (3 if pend_q[0][2] <= 1 else 1):
                            pop_pend()
            while pend_q:
                pop_pend()
            norm_half(1, 1, 0)
            norm_half(1, 1, 1)
            for blk in range(FBS // 128, N // 128):
                proj_blk(blk)


def _build(cfg_key):
    from concourse import bacc, mybir, tile

    cfg = dict(cfg_key)
    dt = mybir.dt
    nc = bacc.Bacc("TRN2", target_bir_lowering=False, debug=False,
                   num_devices=8)
    _d = {"f32": dt.float32, "f32r": dt.float32r, "bf16": dt.bfloat16}
    dt_qkv = _d[cfg["qkv"]]
    dt_proj = _d[cfg["proj"]]
    dt_conv = dt.bfloat16 if cfg["attn"] == "bf16" else dt.float32
    shapes = {
        "xt": ([DIM, N], dt_qkv), "mt": ([256, N], dt_conv),
        "st": ([256, N], dt_conv),
        "wqk": ([DIM, 512], dt_qkv), "wv": ([DIM, 256], dt_qkv),
        "pjt": ([256, DIM], dt_proj),
        "mcw": ([128, 8], dt.float32),
        "scw": ([128, 8], dt.float32),
        "qkb": ([128, 4], dt.float32), "vbv": ([128, 2], dt.float32),
    }
    io = {}
    for name, (shape, dtt) in shapes.items():
        io[name] = nc.dram_tensor(name, shape, dtt,
                                  kind="ExternalInput").ap()
    io["out"] = nc.dram_tensor("out", [N, DIM], dt.float32,
                               kind="ExternalOutput").ap()
    with tile.TileContext(nc) as tc:
        _emit(tc, nc, io, cfg)
    nc.compile()
    return nc


def _get_program(cfg):
    key = tuple(sorted(cfg.items()))
    if key not in _CACHE:
        _CACHE[key] = _build(key)
    return _CACHE[key]


# ------------------------------------------------------------------ wrapper
def kernel(_cfg=None, _want_results=False, **inputs):
    from concourse.bass_utils import run_bass_kernel_spmd

    cfg = dict(_DEFAULT_CFG)
    if _cfg:
        cfg.update(_cfg)
    env_cfg = os.environ.get("BASSKERN_CFG")
    if env_cfg:  # e.g. "attn=f32r,qkv=f32r"
        for kv in env_cfg.split(","):
            k, v = kv.split("=")
            cfg[k] = v

    inputs = {k: np.asarray(v, dtype=np.float32) for k, v in inputs.items()}
    nc = _get_program(cfg)
    in_maps = [_host_prep(core, inputs, cfg) for core in range(8)]
    res = run_bass_kernel_spmd(nc, in_maps, list(range(8)))

    out = np.empty((B, N, DIM), np.float32)
    pb = inputs["proj_b"]
    for b in range(B):
        out[b] = res.results[2 * b]["out"] + res.results[2 * b + 1]["out"] + pb
    if _want_results:
        return out, res
    return out


# revision 26
# speedup vs baseline: 1.4204x; 1.0121x over previous
"""Trainium2 Bass kernel for nn_AttentionWithVQ (B=4, N=2048, DIM=512, H=8,
depthwise-conv positional term, softmax attention, output projection).

Sharding: data-parallel over B (4 batches x 2 core-groups) and tensor-parallel
over heads (4 heads per core) -> 8 cores, fully independent per core except a
final partial-sum over the two head-groups of each batch, done on host at
gather time (the output projection contracts over heads).

Core algorithmic fusion: the score matrix
    S = 0.5*(scale * q @ k^T + scale * conv1(m) @ conv2(s)^T)
is ONE matmul over a concatenated 128-feature axis:
    S = Qp^T @ Kp,  Qp = [q*scale*0.5 ; conv1(m)*scale*0.5], Kp = [k ; conv2(s)]
which exactly fills the 128x128 PE array contraction dim.

Softmax denominators come for free by appending a ones-column to V
(attn@[V|1] yields the row-sums of exp(S) in the last output row); exp() is
numerically safe without max-subtraction for this problem's score magnitudes
(|S| < ~1 given the 0.02-scaled weights).

Partition alignment: compute engines are lane-locked (PSUM partition p ->
SBUF partition p), so per-head feature layouts alternate by head parity
(even heads [qk;conv], odd heads [conv;qk]) making every PSUM->SBUF copy
partition-aligned; the few genuinely shifting copies (odd-head attention
outputs, denominator rows) go through DMA, which can move partitions freely.
All permutation bookkeeping is done host-side in numpy when preparing
per-core inputs.

Schedule (v2): the kernel is a single software-pipelined stream ordered to
keep the PE and ACT engines saturated end-to-end:
  - input DMAs are split across the SP/ACT/Pool queues with the
    qkv-critical tensors (xt, wqk) first;
  - qkv bias-adds run on the ACT engine (idle until the first exp);
  - attention runs stripe-outer (q 1024-blocks) / head-inner, with
    scores(nk+1) emitted before attnV(nk) so exp latency is hidden, the
    v-projection matmuls interleaved into the first head's window and the
    second half of the q/k projection into the second head's window;
  - each stripe is normalized, projected, and DMA'd out as soon as its
    4 heads finish, overlapping the next stripe's attention.
"""

import os
import sys

sys.path.insert(0, "/opt/trn_rl_repo")

import numpy as np

# ---------------------------------------------------------------- constants
B, N, DIM, HEAD, VQE_K = 4, 2048, 512, 8, 3
Dh = DIM // HEAD            # 64
HPC = HEAD // 2             # heads per core (8 cores = 4 batch * 2 groups)
P = 128
FB = 512                    # one fp32 PSUM bank
NKB = N // P                # 16
SCALE_Q = Dh ** -0.5 * 0.5  # folds the 0.5 score scale into the q/conv1 side

_DEFAULT_CFG = {"qkv": "bf16", "attn": "bf16", "proj": "bf16"}

_CACHE = {}


def _np_dt(v):
    if v == "bf16":
        import ml_dtypes
        return ml_dtypes.bfloat16
    return np.float32


# ---------------------------------------------------------------- host prep
def _host_prep(core, inp, cfg=None):
    """Build the per-core input arrays (sharding + layout permutations)."""
    cfg = cfg or _DEFAULT_CFG
    b, g = core // 2, core % 2
    f32 = np.float32
    x, m, s = inp["x"], inp["m"], inp["s"]
    qkv_w, qkv_b = inp["qkv_w"], inp["qkv_b"]
    proj_w = inp["proj_w"]
    p1w = inp["pe1_w"].reshape(HEAD, VQE_K)
    p2w = inp["pe2_w"].reshape(HEAD, VQE_K)
    pe1_b, pe2_b = inp["pe1_b"], inp["pe2_b"]
    dt_qkv = _np_dt(cfg["qkv"])
    dt_proj = _np_dt(cfg["proj"])
    dt_conv = _np_dt("bf16" if cfg["attn"] == "bf16" else "f32")

    d = {}
    d["xt"] = np.ascontiguousarray(x[b].T).astype(dt_qkv)  # [512, 2048]

    # m/s transposed, tile t rows = [head(2t+1) feats ; head(2t) feats]
    mt = np.empty((256, N), f32)
    st = np.empty((256, N), f32)
    mcw = np.zeros((128, 8), f32)
    scw = np.zeros((128, 8), f32)
    for t in range(2):
        h_lo, h_hi = g * 4 + 2 * t + 1, g * 4 + 2 * t
        mt[t * 128:t * 128 + 64] = m[b][:, h_lo * 64:(h_lo + 1) * 64].T
        mt[t * 128 + 64:t * 128 + 128] = m[b][:, h_hi * 64:(h_hi + 1) * 64].T
        st[t * 128:t * 128 + 64] = s[b][:, h_lo * 64:(h_lo + 1) * 64].T
        st[t * 128 + 64:t * 128 + 128] = s[b][:, h_hi * 64:(h_hi + 1) * 64].T
        for p in range(128):
            h = g * 4 + 2 * t + (1 if p < 64 else 0)
            mcw[p, 4 * t:4 * t + 3] = p1w[h] * SCALE_Q
            scw[p, 4 * t:4 * t + 3] = p2w[h]
            mcw[p, 4 * t + 3] = pe1_b[h] * SCALE_Q
            scw[p, 4 * t + 3] = pe2_b[h]
    d["mt"], d["st"] = mt.astype(dt_conv), st.astype(dt_conv)
    d["mcw"], d["scw"] = mcw, scw

    # q/k projection weights: chunk ch=(t, q|k) = [even-head rows; odd-head rows]
    wqk_f = np.empty((512, DIM), f32)
    qkb = np.zeros((128, 4), f32)
    for t in range(2):
        for j in range(2):  # 0=q, 1=k
            ch = 2 * t + j
            h_e, h_o = g * 4 + 2 * t, g * 4 + 2 * t + 1
            base = j * DIM
            wqk_f[ch * 128:ch * 128 + 64] = qkv_w[base + h_e * 64:base + (h_e + 1) * 64]
            wqk_f[ch * 128 + 64:(ch + 1) * 128] = qkv_w[base + h_o * 64:base + (h_o + 1) * 64]
            qkb[0:64, ch] = qkv_b[base + h_e * 64:base + (h_e + 1) * 64]
            qkb[64:128, ch] = qkv_b[base + h_o * 64:base + (h_o + 1) * 64]
            if j == 0:
                wqk_f[ch * 128:(ch + 1) * 128] *= SCALE_Q
                qkb[:, ch] *= SCALE_Q
    d["wqk"] = np.ascontiguousarray(wqk_f.T).astype(dt_qkv)  # [c=512, f=512]
    d["qkb"] = qkb

    d["wv"] = np.ascontiguousarray(
        qkv_w[2 * DIM + g * 256:2 * DIM + (g + 1) * 256].T).astype(dt_qkv)

    # proj rows / v-bias in aT partition order:
    # aT tile t partition p -> head 2t+(p>=64), d=p%64
    pjt = np.empty((256, DIM), f32)
    vbv = np.empty((256,), f32)
    for t in range(2):
        for p in range(128):
            h_l = 2 * t + (1 if p >= 64 else 0)
            h = g * 4 + h_l
            pjt[t * 128 + p] = proj_w[:, h * 64 + (p % 64)]
            vbv[t * 128 + p] = qkv_b[2 * DIM + h * 64 + (p % 64)]
    d["pjt"] = pjt.astype(dt_proj)
    d["vbv"] = np.ascontiguousarray(vbv.reshape(2, 128).T.astype(f32))  # [128, 2]
    return d


# ------------------------------------------------------------- device build
def _emit(tc, nc, io, cfg):
    from contextlib import ExitStack

    from concourse import mybir

    dt = mybir.dt
    f32 = dt.float32
    AF = mybir.ActivationFunctionType
    ALU = mybir.AluOpType

    def _dt(v):
        return {"f32": f32, "f32r": dt.float32r, "bf16": dt.bfloat16}[v]

    dt_qkv = _dt(cfg["qkv"])
    dt_attn = _dt(cfg["attn"])
    dt_proj = _dt(cfg["proj"])
    dt_conv = dt.bfloat16 if dt_attn == dt.bfloat16 else f32
    # exp granularity: one ACT op per FBS-wide stripe (matmuls within are
    # still 512-wide: a matmul output cannot cross a PSUM bank)
    FBS = 1024
    NIH = FBS // FB           # 2
    NQ2 = N // FBS            # 2 q-stripes

    with ExitStack() as ctx:
        persist = ctx.enter_context(tc.tile_pool(name="persist", bufs=1))

        # ---- persistent tiles
        wqk_sb, wv_sb, xt_sb = [], [], []
        QP, KP, v_sb, aT, pjt_sb = [], [], [], [], []
        # critical-path inputs first: xt on the SP queue, wqk on the ACT
        # queue so both stream concurrently from t=0
        # xt split across both HWDGE queues so the qkv c-loop is not gated
        # on one queue streaming all four blocks serially
        for c in range(4):
            xt = persist.tile([128, N], dt_qkv, name=f"xt{c}", tag=f"xt{c}")
            eng = nc.sync if c % 2 == 0 else nc.scalar
            eng.dma_start(xt[:], io["xt"][c * 128:(c + 1) * 128, :])
            xt_sb.append(xt)
        for c in range(4):
            w = persist.tile([128, 512], dt_qkv, name=f"wqk{c}", tag=f"wqk{c}")
            eng = nc.scalar if c % 2 == 0 else nc.sync
            eng.dma_start(w[:], io["wqk"][c * 128:(c + 1) * 128, :])
            wqk_sb.append(w)
        mcw_sb = persist.tile([128, 8], f32, name="mcw", tag="mcw")
        nc.gpsimd.dma_start(mcw_sb[:], io["mcw"][:, :])
        scw_sb = persist.tile([128, 8], f32, name="scw", tag="scw")
        nc.gpsimd.dma_start(scw_sb[:], io["scw"][:, :])
        qkb_sb = persist.tile([128, 4], f32, name="qkb", tag="qkb")
        nc.sync.dma_start(qkb_sb[:], io["qkb"][:, :])

        for h in range(HPC):
            QP.append(persist.tile([128, N], dt_attn, name=f"QP{h}", tag=f"QP{h}"))
            KP.append(persist.tile([128, N], dt_attn, name=f"KP{h}", tag=f"KP{h}"))
        # per-head V block is [v(64) | ones | zero-pad] = 66 columns: matmul
        # operands need 4-byte-aligned offsets, so the block width must be
        # even for 2-byte dtypes (66*h*2 is always 4-aligned)
        for blk in range(NKB):
            v_sb.append(persist.tile([128, HPC * 66], dt_attn, name=f"vsb{blk}",
                                     tag=f"vsb{blk}"))
        for t in range(2):
            aT.append(persist.tile([128, N], dt_proj, name=f"aT{t}", tag=f"aT{t}"))
        # softmax denominators, DMA-reshaped onto 16-partition tiles so the
        # (slow per-element) reciprocal runs 16x wider than a row layout;
        # head h at rows h*32 (32-aligned for the DVE), stripe in columns
        denR = persist.tile([128, 128], f32, name="denR", tag="denR")
        # stationary 0/1 patterns for the denominator-broadcast matmul:
        # lhsT = bcpat[64:65, par*128:(par+1)*128] replicates a [1, FB] den
        # row onto the head's 64-partition half of a PSUM tile (bf16: the
        # BIR verifier requires f32r operands to be produced as f32r, which
        # memset/copy cannot do)
        bcpat = persist.tile([128, 256], dt.bfloat16, name="bcpat",
                             tag="bcpat")
        nc.vector.memset(bcpat[:, 0:64], 1.0)
        nc.vector.memset(bcpat[:, 64:192], 0.0)
        nc.vector.memset(bcpat[:, 192:256], 1.0)

        for c in range(4):
            w = persist.tile([128, 256], dt_qkv, name=f"wv{c}", tag=f"wv{c}")
            nc.scalar.dma_start(w[:], io["wv"][c * 128:(c + 1) * 128, :])
            wv_sb.append(w)
        vbv_sb = persist.tile([128, 2], f32, name="vbv", tag="vbv")
        nc.sync.dma_start(vbv_sb[:], io["vbv"][:, :])
        for f in range(2):
            w = persist.tile([128, 512], dt_proj, name=f"pjt{f}", tag=f"pjt{f}")
            nc.scalar.dma_start(w[:], io["pjt"][f * 128:(f + 1) * 128, :])
            pjt_sb.append(w)

        # ---- depthwise convs (DVE, bf16 2x mode; inputs on the Pool queue)
        # PE p-state warm-up: ~10 back-to-back dummy matmuls on the (tiny,
        # already-memset) bcpat tile during the input-DMA wait, so the real
        # qkv matmuls start at full clock instead of ramping through them
        with tc.tile_pool(name="ps_warm", bufs=1, space="PSUM") as ps_wp:
            wps = ps_wp.tile([128, 256], f32, name="warm", tag="warm")
            for _ in range(10):
                nc.tensor.matmul(wps[:], bcpat[:, 0:128], bcpat[:, 0:256],
                                 start=True, stop=True)

        # t=0 convs first: heads 0/1 gate the first attention window
        with tc.tile_pool(name="conv", bufs=2) as convp:
            for t in range(2):
                for src, wv_, dst in (("mt", mcw_sb, QP), ("st", scw_sb, KP)):
                    xin = convp.tile([128, N], dt_conv, name=f"ci_{src}{t}",
                                     tag="cin")
                    nc.gpsimd.dma_start(xin[:], io[src][t * 128:(t + 1) * 128, :])
                    y = convp.tile([128, N], dt_conv, name=f"cy_{src}{t}",
                                   tag="cy", bufs=1)
                    w0, w1, w2, cb = (wv_[:, 4 * t + k:4 * t + k + 1]
                                      for k in range(4))
                    nc.vector.tensor_scalar(y[:], xin[:], w1, cb,
                                            ALU.mult, ALU.add)
                    nc.vector.scalar_tensor_tensor(
                        y[:, 1:], xin[:, :N - 1], w0, y[:, 1:],
                        ALU.mult, ALU.add)
                    nc.vector.scalar_tensor_tensor(
                        y[:, :N - 1], xin[:, 1:], w2, y[:, :N - 1],
                        ALU.mult, ALU.add)
                    nc.vector.tensor_copy(dst[2 * t + 1][0:64, :], y[0:64, :])
                    nc.vector.tensor_copy(dst[2 * t][64:128, :], y[64:128, :])

        # ---- q/k projections, first half (chunks 0,1 = heads 0,1).
        # Bias-adds run on ACT (idle until the first exp); the PSUM pool
        # closes before attention so its banks are reusable there.
        def qk_chunk_matmuls(ps, ch, qb):
            for ih in range(NIH):
                hqs = slice(qb * FBS + ih * FB, qb * FBS + (ih + 1) * FB)
                for c in range(4):
                    nc.tensor.matmul(
                        ps[:, ih * FB:(ih + 1) * FB],
                        wqk_sb[c][:, ch * 128:(ch + 1) * 128],
                        xt_sb[c][:, hqs],
                        start=(c == 0), stop=(c == 3))

        def qk_bias_out(ps, ch, qb, engine):
            t = ch // 2
            qs = slice(qb * FBS, (qb + 1) * FBS)
            dst = QP if ch % 2 == 0 else KP
            if engine == "act":
                nc.scalar.activation(dst[2 * t][0:64, qs], ps[0:64, :],
                                     AF.Identity, bias=qkb_sb[0:64, ch:ch + 1])
                nc.scalar.activation(dst[2 * t + 1][64:128, qs], ps[64:128, :],
                                     AF.Identity,
                                     bias=qkb_sb[64:128, ch:ch + 1])
            else:
                nc.vector.tensor_scalar_add(dst[2 * t][0:64, qs], ps[0:64, :],
                                            qkb_sb[0:64, ch:ch + 1])
                nc.vector.tensor_scalar_add(dst[2 * t + 1][64:128, qs],
                                            ps[64:128, :],
                                            qkb_sb[64:128, ch:ch + 1])

        with tc.tile_pool(name="ps_qkv", bufs=1, space="PSUM") as ps_qkp:
            for ch in range(4):
                for qb in range(NQ2):
                    ps = ps_qkp.tile([128, FBS], f32, name="psqk", tag="psqk",
                                     bufs=3)
                    qk_chunk_matmuls(ps, ch, qb)
                    qk_bias_out(ps, ch, qb, "act")

        # ---- v projection (needed before the first head's attnV)
        with tc.tile_pool(name="ps_v", bufs=1, space="PSUM") as ps_vp:
            for blk in range(NKB):
                bs = slice(blk * 128, (blk + 1) * 128)
                ps = ps_vp.tile([128, 256], f32, name="psv", tag="psv",
                                bufs=2)
                for c in range(4):
                    nc.tensor.matmul(ps[:], xt_sb[c][:, bs], wv_sb[c][:],
                                     start=(c == 0), stop=(c == 3))
                v3 = v_sb[blk].rearrange("p (h f) -> p h f", h=HPC)
                nc.vector.tensor_copy(v3[:, :, 0:64],
                                      ps.rearrange("p (h f) -> p h f", h=HPC))
                # memset lacks float32r support; write the ones/pad columns
                # through an f32 view (identical bit pattern)
                ones_ap, pad_ap = v3[:, :, 64:65], v3[:, :, 65:66]
                if dt_attn == dt.float32r:
                    ones_ap = ones_ap.bitcast(f32)
                    pad_ap = pad_ap.bitcast(f32)
                nc.vector.memset(ones_ap, 1.0)
                nc.vector.memset(pad_ap, 0.0)

        # ---- attention: one flat software-pipelined (q2, h, nk) stream.
        # attnV lags scores/exp by one step so the pipeline never drains at
        # head boundaries; head drains, denominator normalizes, and the
        # stripe projections are injected as fill steps inside later head
        # windows to keep the PE stream dense.
        # PSUM budget (8 banks): s_ps 2x2 + o_ps 1x2 + pj/bc 2x1.
        with tc.tile_pool(name="ps_s", bufs=2, space="PSUM") as ps_sp, \
                tc.tile_pool(name="ps_o", bufs=1, space="PSUM") as ps_op, \
                tc.tile_pool(name="ps_pj", bufs=2, space="PSUM") as ps_pjp, \
                tc.tile_pool(name="esbp", bufs=4) as esbp, \
                tc.tile_pool(name="stg", bufs=2) as stgp, \
                tc.tile_pool(name="osbp", bufs=3) as osbp:
            o_tiles, stgd_t = {}, {}

            def attn_v(q2, h, nk, e_sb):
                # lazy o_ps alloc: first write lands after the previous
                # head's last attnV is already emitted (bufs=1 slot reuse)
                if nk == 0:
                    o_tiles[(q2, h)] = ps_op.tile([66, FBS], f32,
                                                  name=f"ops{h}_{q2}",
                                                  tag="ops")
                o_ps = o_tiles[(q2, h)]
                vcols = slice(h * 66, (h + 1) * 66)
                for ih in range(NIH):
                    nc.tensor.matmul(
                        o_ps[:, ih * FB:(ih + 1) * FB],
                        v_sb[nk][:, vcols],
                        e_sb[:, ih * FB:(ih + 1) * FB],
                        start=(nk == 0), stop=(nk == NKB - 1))

            def drain_head(q2, h):
                """aT copies + reciprocal of the denominator row."""
                t, odd = h // 2, h % 2
                cs = slice(q2 * FBS, (q2 + 1) * FBS)
                o_ps = o_tiles[(q2, h)]
                # lane-locked engines cannot shift partitions and DMA
                # cannot read PSUM, so shifting copies stage through SBUF
                stgd = stgp.tile([65, FBS], f32, name=f"sd{h}_{q2}",
                                 tag="stgd")
                if odd:
                    stg = stgp.tile([64, FBS], dt_proj, name=f"sg{h}_{q2}",
                                    tag="stg")
                    nc.vector.tensor_copy(stg[:], o_ps[0:64, :])
                    nc.sync.dma_start(aT[t][64:128, cs], stg[:])
                else:
                    nc.vector.tensor_copy(aT[t][0:64, cs], o_ps[0:64, :])
                nc.vector.tensor_copy(stgd[64:65, :], o_ps[64:65, :])
                # den row -> denR [16, 64] block (rows h*32, cols q2*64):
                # reciprocal, DMA-reshape back to a row, cast to bf16
                dblk = denR[h * 32:h * 32 + 16, q2 * 64:(q2 + 1) * 64]
                nc.sync.dma_start(dblk, stgd[64:65, :])
                nc.vector.reciprocal(dblk, dblk)
                sdf = stgp.tile([65, FBS], f32, name=f"sf{h}_{q2}",
                                tag="sdf")
                nc.sync.dma_start(sdf[64:65, :], dblk)
                sdb = stgp.tile([65, FBS], dt.bfloat16, name=f"sb{h}_{q2}",
                                tag="sdb")
                nc.vector.tensor_copy(sdb[64:65, :], sdf[64:65, :])
                stgd_t[(q2, h)] = sdb

            def norm_half(q2, t, half):
                """PE-broadcast the 1/den rows of heads 2t,2t+1 and scale."""
                cols = slice(q2 * FBS + half * FB, q2 * FBS + (half + 1) * FB)
                bc = ps_pjp.tile([128, FB], f32, name="bc", tag="pj")
                for par in range(2):
                    sd = stgd_t[(q2, 2 * t + par)]
                    nc.tensor.matmul(
                        bc[:],
                        bcpat[64:65, par * 128:(par + 1) * 128],
                        sd[64:65, half * FB:(half + 1) * FB],
                        start=(par == 0), stop=(par == 1))
                nc.vector.tensor_mul(aT[t][:, cols], aT[t][:, cols], bc[:])
                nc.vector.tensor_scalar_add(aT[t][:, cols], aT[t][:, cols],
                                            vbv_sb[:, t:t + 1])

            def proj_blk(blk):
                bs = slice(blk * 128, (blk + 1) * 128)
                pj = ps_pjp.tile([128, FB], f32, name="pj", tag="pj")
                for f in range(2):
                    nc.tensor.matmul(pj[:], aT[f][:, bs], pjt_sb[f][:],
                                     start=(f == 0), stop=(f == 1))
                ob = osbp.tile([128, FB], f32, name="ob", tag="ob")
                nc.vector.tensor_copy(ob[:], pj[:])
                eng = nc.gpsimd if blk % 2 == 0 else nc.sync
                eng.dma_start(io["out"][bs, :], ob[:])

            # fill work injected at (q2, h, nk) steps: normalizes once both
            # heads of a t-group drained (nk>=4: the reciprocal chain takes
            # ~4us after the drain pops at nk=0), stripe-0 projection spread
            # one block per two steps across stripe 1's first two windows
            fills = {
                (0, 2, 4): [lambda: norm_half(0, 0, 0)],
                (0, 2, 5): [lambda: norm_half(0, 0, 1)],
                (1, 0, 4): [lambda: norm_half(0, 1, 0)],
                (1, 0, 5): [lambda: norm_half(0, 1, 1)],
                (1, 2, 4): [lambda: norm_half(1, 0, 0)],
                (1, 2, 5): [lambda: norm_half(1, 0, 1)],
            }
            proj_steps = [(1, 0, 7), (1, 0, 9), (1, 0, 11), (1, 0, 13),
                          (1, 0, 15), (1, 1, 1), (1, 1, 3), (1, 1, 5)]
            for i, step in enumerate(proj_steps):
                fills.setdefault(step, []).append(lambda blk=i: proj_blk(blk))

            # attnV lags scores/exp via a queue: lag 3 across a head start
            # (so the previous head's o_ps drain — bufs=1 — never stalls the
            # PE), catching back up to lag 1 within the head
            pend_q = []

            def pop_pend():
                q2p, hp, nkp, e = pend_q.pop(0)
                attn_v(q2p, hp, nkp, e)
                if nkp == NKB - 1:
                    drain_head(q2p, hp)

            for q2 in range(NQ2):
                qbase = q2 * FBS
                for h in range(HPC):
                    for nk in range(NKB):
                        for f in fills.get((q2, h, nk), ()):
                            f()
                        ks = slice(nk * 128, (nk + 1) * 128)
                        s_ps = ps_sp.tile([128, FBS], f32, name="sps",
                                          tag="sps")
                        for ih in range(NIH):
                            hqs = slice(qbase + ih * FB, qbase + (ih + 1) * FB)
                            nc.tensor.matmul(s_ps[:, ih * FB:(ih + 1) * FB],
                                             KP[h][:, ks], QP[h][:, hqs],
                                             start=True, stop=True)
                        e_sb = esbp.tile([128, FBS], dt_attn, name="esb",
                                         tag="esb")
                        nc.scalar.activation(e_sb[:], s_ps[:], AF.Exp)
                        pend_q.append((q2, h, nk, e_sb))
                        while len(pend_q) > (3 if pend_q[0][2] <= 1 else 1):
                            pop_pend()
            while pend_q:
                pop_pend()
            norm_half(1, 1, 0)
            norm_half(1, 1, 1)
            for blk in range(FBS // 128, N // 128):
                proj_blk(blk)


def _build(cfg_key):
    from concourse import bacc, mybir, tile

    cfg = dict(cfg_key)
    dt = mybir.dt
    nc = bacc.Bacc("TRN2", target_bir_lowering=False, debug=False,
                   num_devices=8)
    _d = {"f32": dt.float32, "f32r": dt.float32r, "bf16": dt.bfloat16}
    dt_qkv = _d[cfg["qkv"]]
    dt_proj = _d[cfg["proj"]]
    dt_conv = dt.bfloat16 if cfg["attn"] == "bf16" else dt.float32
    shapes = {
        "xt": ([DIM, N], dt_qkv), "mt": ([256, N], dt_conv),
        "st": ([256, N], dt_conv),
        "wqk": ([DIM, 512], dt_qkv), "wv": ([DIM, 256], dt_qkv),
        "pjt": ([256, DIM], dt_proj),
        "mcw": ([128, 8], dt.float32),
        "scw": ([128, 8], dt.float32),
        "qkb": ([128, 4], dt.float32), "vbv": ([128, 2], dt.float32),
    }
    io = {}
    for name, (shape, dtt) in shapes.items():
        io[name] = nc.dram_tensor(name, shape, dtt,
                                  kind="ExternalInput").ap()
    io["out"] = nc.dram_tensor("out", [N, DIM], dt.float32,
                               kind="ExternalOutput").ap()
    with tile.TileContext(nc) as tc:
        _emit(tc, nc, io, cfg)
    nc.compile()
    return nc


def _get_program(cfg):
    key = tuple(sorted(cfg.items()))
    if key not in _CACHE:
        _CACHE[key] = _build(key)
    return _CACHE[key]


# ------------------------------------------------------------------ wrapper
def kernel(_cfg=None, _want_results=False, **inputs):
    from concourse.bass_utils import run_bass_kernel_spmd

    cfg = dict(_DEFAULT_CFG)
    if _cfg:
        cfg.update(_cfg)
    env_cfg = os.environ.get("BASSKERN_CFG")
    if env_cfg:  # e.g. "attn=f32r,qkv=f32r"
        for kv in env_cfg.split(","):
            k, v = kv.split("=")
            cfg[k] = v

    inputs = {k: np.asarray(v, dtype=np.float32) for k, v in inputs.items()}
    nc = _get_program(cfg)
    in_maps = [_host_prep(core, inputs, cfg) for core in range(8)]
    res = run_bass_kernel_spmd(nc, in_maps, list(range(8)))

    out = np.empty((B, N, DIM), np.float32)
    pb = inputs["proj_b"]
    for b in range(B):
        out[b] = res.results[2 * b]["out"] + res.results[2 * b + 1]["out"] + pb
    if _want_results:
        return out, res
    return out


# revision 36
# speedup vs baseline: 1.4255x; 1.0036x over previous
"""Trainium2 Bass kernel for nn_AttentionWithVQ (B=4, N=2048, DIM=512, H=8,
depthwise-conv positional term, softmax attention, output projection).

Sharding: data-parallel over B (4 batches x 2 core-groups) and tensor-parallel
over heads (4 heads per core) -> 8 cores, fully independent per core except a
final partial-sum over the two head-groups of each batch, done on host at
gather time (the output projection contracts over heads).

Core algorithmic fusion: the score matrix
    S = 0.5*(scale * q @ k^T + scale * conv1(m) @ conv2(s)^T)
is ONE matmul over a concatenated 128-feature axis:
    S = Qp^T @ Kp,  Qp = [q*scale*0.5 ; conv1(m)*scale*0.5], Kp = [k ; conv2(s)]
which exactly fills the 128x128 PE array contraction dim.

Softmax denominators come for free by appending a ones-column to V
(attn@[V|1] yields the row-sums of exp(S) in the last output row); exp() is
numerically safe without max-subtraction for this problem's score magnitudes
(|S| < ~1 given the 0.02-scaled weights).

Partition alignment: compute engines are lane-locked (PSUM partition p ->
SBUF partition p), so per-head feature layouts alternate by head parity
(even heads [qk;conv], odd heads [conv;qk]) making every PSUM->SBUF copy
partition-aligned; the few genuinely shifting copies (odd-head attention
outputs, denominator rows) go through DMA, which can move partitions freely.
All permutation bookkeeping is done host-side in numpy when preparing
per-core inputs.

Schedule (v2): the kernel is a single software-pipelined stream ordered to
keep the PE and ACT engines saturated end-to-end:
  - input DMAs are split across the SP/ACT/Pool queues with the
    qkv-critical tensors (xt, wqk) first;
  - qkv bias-adds run on the ACT engine (idle until the first exp);
  - attention runs stripe-outer (q 1024-blocks) / head-inner, with
    scores(nk+1) emitted before attnV(nk) so exp latency is hidden, the
    v-projection matmuls interleaved into the first head's window and the
    second half of the q/k projection into the second head's window;
  - each stripe is normalized, projected, and DMA'd out as soon as its
    4 heads finish, overlapping the next stripe's attention.
"""

import os
import sys

sys.path.insert(0, "/opt/trn_rl_repo")

import numpy as np

# ---------------------------------------------------------------- constants
B, N, DIM, HEAD, VQE_K = 4, 2048, 512, 8, 3
Dh = DIM // HEAD            # 64
HPC = HEAD // 2             # heads per core (8 cores = 4 batch * 2 groups)
P = 128
FB = 512                    # one fp32 PSUM bank
NKB = N // P                # 16
SCALE_Q = Dh ** -0.5 * 0.5  # folds the 0.5 score scale into the q/conv1 side

_DEFAULT_CFG = {"qkv": "bf16", "attn": "bf16", "proj": "bf16", "av": "same"}

_CACHE = {}


def _np_dt(v):
    if v == "bf16":
        import ml_dtypes
        return ml_dtypes.bfloat16
    return np.float32


# ---------------------------------------------------------------- host prep
def _host_prep(core, inp, cfg=None):
    """Build the per-core input arrays (sharding + layout permutations)."""
    cfg = cfg or _DEFAULT_CFG
    b, g = core // 2, core % 2
    f32 = np.float32
    x, m, s = inp["x"], inp["m"], inp["s"]
    qkv_w, qkv_b = inp["qkv_w"], inp["qkv_b"]
    proj_w = inp["proj_w"]
    p1w = inp["pe1_w"].reshape(HEAD, VQE_K)
    p2w = inp["pe2_w"].reshape(HEAD, VQE_K)
    pe1_b, pe2_b = inp["pe1_b"], inp["pe2_b"]
    dt_qkv = _np_dt(cfg["qkv"])
    dt_proj = _np_dt(cfg["proj"])
    dt_conv = _np_dt("bf16" if cfg["attn"] == "bf16" else "f32")

    d = {}
    d["xt"] = np.ascontiguousarray(x[b].T).astype(dt_qkv)  # [512, 2048]

    # m/s transposed, tile t rows = [head(2t+1) feats ; head(2t) feats]
    mt = np.empty((256, N), f32)
    st = np.empty((256, N), f32)
    mcw = np.zeros((128, 8), f32)
    scw = np.zeros((128, 8), f32)
    for t in range(2):
        h_lo, h_hi = g * 4 + 2 * t + 1, g * 4 + 2 * t
        mt[t * 128:t * 128 + 64] = m[b][:, h_lo * 64:(h_lo + 1) * 64].T
        mt[t * 128 + 64:t * 128 + 128] = m[b][:, h_hi * 64:(h_hi + 1) * 64].T
        st[t * 128:t * 128 + 64] = s[b][:, h_lo * 64:(h_lo + 1) * 64].T
        st[t * 128 + 64:t * 128 + 128] = s[b][:, h_hi * 64:(h_hi + 1) * 64].T
        for p in range(128):
            h = g * 4 + 2 * t + (1 if p < 64 else 0)
            mcw[p, 4 * t:4 * t + 3] = p1w[h] * SCALE_Q
            scw[p, 4 * t:4 * t + 3] = p2w[h]
            mcw[p, 4 * t + 3] = pe1_b[h] * SCALE_Q
            scw[p, 4 * t + 3] = pe2_b[h]
    d["mt"], d["st"] = mt.astype(dt_conv), st.astype(dt_conv)
    d["mcw"], d["scw"] = mcw, scw

    # q/k projection weights: chunk ch=(t, q|k) = [even-head rows; odd-head rows]
    wqk_f = np.empty((512, DIM), f32)
    qkb = np.zeros((128, 4), f32)
    for t in range(2):
        for j in range(2):  # 0=q, 1=k
            ch = 2 * t + j
            h_e, h_o = g * 4 + 2 * t, g * 4 + 2 * t + 1
            base = j * DIM
            wqk_f[ch * 128:ch * 128 + 64] = qkv_w[base + h_e * 64:base + (h_e + 1) * 64]
            wqk_f[ch * 128 + 64:(ch + 1) * 128] = qkv_w[base + h_o * 64:base + (h_o + 1) * 64]
            qkb[0:64, ch] = qkv_b[base + h_e * 64:base + (h_e + 1) * 64]
            qkb[64:128, ch] = qkv_b[base + h_o * 64:base + (h_o + 1) * 64]
            if j == 0:
                wqk_f[ch * 128:(ch + 1) * 128] *= SCALE_Q
                qkb[:, ch] *= SCALE_Q
    d["wqk"] = np.ascontiguousarray(wqk_f.T).astype(dt_qkv)  # [c=512, f=512]
    d["qkb"] = qkb

    d["wv"] = np.ascontiguousarray(
        qkv_w[2 * DIM + g * 256:2 * DIM + (g + 1) * 256].T).astype(dt_qkv)

    # proj rows / v-bias in aT partition order:
    # aT tile t partition p -> head 2t+(p>=64), d=p%64
    pjt = np.empty((256, DIM), f32)
    vbv = np.empty((256,), f32)
    for t in range(2):
        for p in range(128):
            h_l = 2 * t + (1 if p >= 64 else 0)
            h = g * 4 + h_l
            pjt[t * 128 + p] = proj_w[:, h * 64 + (p % 64)]
            vbv[t * 128 + p] = qkv_b[2 * DIM + h * 64 + (p % 64)]
    d["pjt"] = pjt.astype(dt_proj)
    d["vbv"] = np.ascontiguousarray(vbv.reshape(2, 128).T.astype(f32))  # [128, 2]
    return d


# ------------------------------------------------------------- device build
def _emit(tc, nc, io, cfg):
    from contextlib import ExitStack

    from concourse import mybir

    dt = mybir.dt
    f32 = dt.float32
    AF = mybir.ActivationFunctionType
    ALU = mybir.AluOpType

    def _dt(v):
        return {"f32": f32, "f32r": dt.float32r, "bf16": dt.bfloat16}[v]

    dt_qkv = _dt(cfg["qkv"])
    dt_attn = _dt(cfg["attn"])
    dt_proj = _dt(cfg["proj"])
    dt_conv = dt.bfloat16 if dt_attn == dt.bfloat16 else f32
    # av=f8: exp writes fp8e4m3 and attnV runs DoubleRow (K=256/instr,
    # 2x PE throughput); scores/QP/KP stay in dt_attn
    AV8 = cfg.get("av") == "f8"
    dt_av = dt.float8e4 if AV8 else dt_attn
    VW = 68 if AV8 else 66  # per-head V block width (4B-aligned offsets)
    # exp granularity: one ACT op per FBS-wide stripe (matmuls within are
    # still 512-wide: a matmul output cannot cross a PSUM bank)
    FBS = 1024
    NIH = FBS // FB           # 2
    NQ2 = N // FBS            # 2 q-stripes

    with ExitStack() as ctx:
        persist = ctx.enter_context(tc.tile_pool(name="persist", bufs=1))

        # ---- persistent tiles
        wqk_sb, wv_sb, xt_sb = [], [], []
        QP, KP, v_sb, aT, pjt_sb = [], [], [], [], []
        # critical-path inputs first: xt on the SP queue, wqk on the ACT
        # queue so both stream concurrently from t=0
        # xt split across both HWDGE queues so the qkv c-loop is not gated
        # on one queue streaming all four blocks serially
        for c in range(4):
            xt = persist.tile([128, N], dt_qkv, name=f"xt{c}", tag=f"xt{c}")
            eng = nc.sync if c % 2 == 0 else nc.scalar
            eng.dma_start(xt[:], io["xt"][c * 128:(c + 1) * 128, :])
            xt_sb.append(xt)
        for c in range(4):
            w = persist.tile([128, 512], dt_qkv, name=f"wqk{c}", tag=f"wqk{c}")
            eng = nc.scalar if c % 2 == 0 else nc.sync
            eng.dma_start(w[:], io["wqk"][c * 128:(c + 1) * 128, :])
            wqk_sb.append(w)
        mcw_sb = persist.tile([128, 8], f32, name="mcw", tag="mcw")
        nc.gpsimd.dma_start(mcw_sb[:], io["mcw"][:, :])
        scw_sb = persist.tile([128, 8], f32, name="scw", tag="scw")
        nc.gpsimd.dma_start(scw_sb[:], io["scw"][:, :])
        qkb_sb = persist.tile([128, 4], f32, name="qkb", tag="qkb")
        nc.sync.dma_start(qkb_sb[:], io["qkb"][:, :])

        for h in range(HPC):
            QP.append(persist.tile([128, N], dt_attn, name=f"QP{h}", tag=f"QP{h}"))
            KP.append(persist.tile([128, N], dt_attn, name=f"KP{h}", tag=f"KP{h}"))
        # per-head V block is [v(64) | ones | zero-pad] = VW columns: matmul
        # operands need 4-byte-aligned offsets, so the block width must keep
        # h*VW*dtsize 4-aligned (66 for 2-byte dtypes, 68 for fp8)
        if AV8:
            for p in range(NKB // 2):
                v_sb.append(persist.tile([128, 2 * HPC * VW], dt_av,
                                         name=f"vsb{p}", tag=f"vsb{p}"))
        else:
            for blk in range(NKB):
                v_sb.append(persist.tile([128, HPC * VW], dt_attn,
                                         name=f"vsb{blk}", tag=f"vsb{blk}"))
        for t in range(2):
            aT.append(persist.tile([128, N], dt_proj, name=f"aT{t}", tag=f"aT{t}"))
        # softmax denominators, DMA-reshaped onto 16-partition tiles so the
        # (slow per-element) reciprocal runs 16x wider than a row layout;
        # head h at rows h*32 (32-aligned for the DVE), stripe in columns
        denR = persist.tile([128, 128], f32, name="denR", tag="denR")
        # stationary 0/1 patterns for the denominator-broadcast matmul:
        # lhsT = bcpat[64:65, par*128:(par+1)*128] replicates a [1, FB] den
        # row onto the head's 64-partition half of a PSUM tile (bf16: the
        # BIR verifier requires f32r operands to be produced as f32r, which
        # memset/copy cannot do)
        bcpat = persist.tile([128, 256], dt.bfloat16, name="bcpat",
                             tag="bcpat")
        nc.vector.memset(bcpat[:, 0:64], 1.0)
        nc.vector.memset(bcpat[:, 64:192], 0.0)
        nc.vector.memset(bcpat[:, 192:256], 1.0)

        for c in range(4):
            w = persist.tile([128, 256], dt_qkv, name=f"wv{c}", tag=f"wv{c}")
            nc.scalar.dma_start(w[:], io["wv"][c * 128:(c + 1) * 128, :])
            wv_sb.append(w)
        vbv_sb = persist.tile([128, 2], f32, name="vbv", tag="vbv")
        nc.sync.dma_start(vbv_sb[:], io["vbv"][:, :])
        for f in range(2):
            w = persist.tile([128, 512], dt_proj, name=f"pjt{f}", tag=f"pjt{f}")
            nc.scalar.dma_start(w[:], io["pjt"][f * 128:(f + 1) * 128, :])
            pjt_sb.append(w)

        # ---- depthwise convs (DVE, bf16 2x mode; inputs on the Pool queue)
        # PE p-state warm-up: ~10 back-to-back dummy matmuls on the (tiny,
        # already-memset) bcpat tile during the input-DMA wait, so the real
        # qkv matmuls start at full clock instead of ramping through them
        with tc.tile_pool(name="ps_warm", bufs=1, space="PSUM") as ps_wp:
            wps = ps_wp.tile([128, 256], f32, name="warm", tag="warm")
            for _ in range(10):
                nc.tensor.matmul(wps[:], bcpat[:, 0:128], bcpat[:, 0:256],
                                 start=True, stop=True)

        # t=0 convs first: heads 0/1 gate the first attention window
        with tc.tile_pool(name="conv", bufs=2) as convp:
            for t in range(2):
                for src, wv_, dst in (("mt", mcw_sb, QP), ("st", scw_sb, KP)):
                    xin = convp.tile([128, N], dt_conv, name=f"ci_{src}{t}",
                                     tag="cin")
                    nc.gpsimd.dma_start(xin[:], io[src][t * 128:(t + 1) * 128, :])
                    y = convp.tile([128, N], dt_conv, name=f"cy_{src}{t}",
                                   tag="cy", bufs=1)
                    w0, w1, w2, cb = (wv_[:, 4 * t + k:4 * t + k + 1]
                                      for k in range(4))
                    nc.vector.tensor_scalar(y[:], xin[:], w1, cb,
                                            ALU.mult, ALU.add)
                    nc.vector.scalar_tensor_tensor(
                        y[:, 1:], xin[:, :N - 1], w0, y[:, 1:],
                        ALU.mult, ALU.add)
                    nc.vector.scalar_tensor_tensor(
                        y[:, :N - 1], xin[:, 1:], w2, y[:, :N - 1],
                        ALU.mult, ALU.add)
                    nc.vector.tensor_copy(dst[2 * t + 1][0:64, :], y[0:64, :])
                    nc.vector.tensor_copy(dst[2 * t][64:128, :], y[64:128, :])

        # ---- q/k projections, first half (chunks 0,1 = heads 0,1).
        # Bias-adds run on ACT (idle until the first exp); the PSUM pool
        # closes before attention so its banks are reusable there.
        def qk_chunk_matmuls(ps, ch, qb):
            for ih in range(NIH):
                hqs = slice(qb * FBS + ih * FB, qb * FBS + (ih + 1) * FB)
                for c in range(4):
                    nc.tensor.matmul(
                        ps[:, ih * FB:(ih + 1) * FB],
                        wqk_sb[c][:, ch * 128:(ch + 1) * 128],
                        xt_sb[c][:, hqs],
                        start=(c == 0), stop=(c == 3))

        def qk_bias_out(ps, ch, qb, engine):
            t = ch // 2
            qs = slice(qb * FBS, (qb + 1) * FBS)
            dst = QP if ch % 2 == 0 else KP
            if engine == "act":
                nc.scalar.activation(dst[2 * t][0:64, qs], ps[0:64, :],
                                     AF.Identity, bias=qkb_sb[0:64, ch:ch + 1])
                nc.scalar.activation(dst[2 * t + 1][64:128, qs], ps[64:128, :],
                                     AF.Identity,
                                     bias=qkb_sb[64:128, ch:ch + 1])
            else:
                nc.vector.tensor_scalar_add(dst[2 * t][0:64, qs], ps[0:64, :],
                                            qkb_sb[0:64, ch:ch + 1])
                nc.vector.tensor_scalar_add(dst[2 * t + 1][64:128, qs],
                                            ps[64:128, :],
                                            qkb_sb[64:128, ch:ch + 1])

        # c-OUTER accumulation over 4 live PSUM tiles (8 banks): matmuls for
        # xt block c start as soon as that block's DMA lands instead of
        # waiting for all four
        with tc.tile_pool(name="ps_qkv", bufs=1, space="PSUM") as ps_qkp:
            for grp in range(2):
                chunks = [(2 * grp + j, qb) for j in range(2)
                          for qb in range(NQ2)]
                tiles = [ps_qkp.tile([128, FBS], f32, name="psqk",
                                     tag="psqk", bufs=4) for _ in chunks]
                for c in range(4):
                    for ps, (ch, qb) in zip(tiles, chunks):
                        for ih in range(NIH):
                            hqs = slice(qb * FBS + ih * FB,
                                        qb * FBS + (ih + 1) * FB)
                            nc.tensor.matmul(
                                ps[:, ih * FB:(ih + 1) * FB],
                                wqk_sb[c][:, ch * 128:(ch + 1) * 128],
                                xt_sb[c][:, hqs],
                                start=(c == 0), stop=(c == 3))
                for ps, (ch, qb) in zip(tiles, chunks):
                    qk_bias_out(ps, ch, qb, "act")

        # ---- v projection (needed before the first head's attnV)
        with tc.tile_pool(name="ps_v", bufs=1, space="PSUM") as ps_vp:
            for blk in range(NKB):
                bs = slice(blk * 128, (blk + 1) * 128)
                ps = ps_vp.tile([128, 256], f32, name="psv", tag="psv",
                                bufs=2)
                for c in range(4):
                    nc.tensor.matmul(ps[:], xt_sb[c][:, bs], wv_sb[c][:],
                                     start=(c == 0), stop=(c == 3))
                if AV8:
                    base = (blk % 2) * HPC * VW
                    vt = v_sb[blk // 2][:, base:base + HPC * VW]
                else:
                    vt = v_sb[blk][:]
                v3 = vt.rearrange("p (h f) -> p h f", h=HPC)
                nc.vector.tensor_copy(v3[:, :, 0:64],
                                      ps.rearrange("p (h f) -> p h f", h=HPC))
                # memset lacks float32r support; write the ones/pad columns
                # through an f32 view (identical bit pattern)
                ones_ap, pad_ap = v3[:, :, 64:65], v3[:, :, 65:VW]
                if dt_attn == dt.float32r:
                    ones_ap = ones_ap.bitcast(f32)
                    pad_ap = pad_ap.bitcast(f32)
                nc.vector.memset(ones_ap, 1.0)
                nc.vector.memset(pad_ap, 0.0)

        # ---- attention: one flat software-pipelined (q2, h, nk) stream.
        # attnV lags scores/exp by one step so the pipeline never drains at
        # head boundaries; head drains, denominator normalizes, and the
        # stripe projections are injected as fill steps inside later head
        # windows to keep the PE stream dense.
        # PSUM budget (8 banks): s_ps 2x2 + o_ps 1x2 + pj/bc 2x1.
        with tc.tile_pool(name="ps_s", bufs=2, space="PSUM") as ps_sp, \
                tc.tile_pool(name="ps_o", bufs=1, space="PSUM") as ps_op, \
                tc.tile_pool(name="ps_pj", bufs=2, space="PSUM") as ps_pjp, \
                tc.tile_pool(name="esbp", bufs=4) as esbp, \
                tc.tile_pool(name="stg", bufs=2) as stgp, \
                tc.tile_pool(name="osbp", bufs=3) as osbp:
            o_tiles, stgd_t = {}, {}

            def attn_v(q2, h, nk, e_sb):
                # lazy o_ps alloc: first write lands after the previous
                # head's last attnV is already emitted (bufs=1 slot reuse)
                if nk == 0:
                    o_tiles[(q2, h)] = ps_op.tile([VW if AV8 else 66, FBS],
                                                  f32, name=f"ops{h}_{q2}",
                                                  tag="ops")
                o_ps = o_tiles[(q2, h)]
                if AV8:
                    # DoubleRow: nk indexes an e/v PAIR (K = 2x128 per call)
                    lhsT = v_sb[nk].rearrange("p (j f) -> p j f",
                                              j=2)[:, :, h * VW:(h + 1) * VW]
                    e3 = e_sb.rearrange("p (j f) -> p j f", j=2)
                    for ih in range(NIH):
                        nc.tensor.matmul(
                            o_ps[:, ih * FB:(ih + 1) * FB],
                            lhsT, e3[:, :, ih * FB:(ih + 1) * FB],
                            start=(nk == 0), stop=(nk == NKB // 2 - 1),
                            perf_mode=mybir.MatmulPerfMode.DoubleRow)
                    return
                vcols = slice(h * VW, (h + 1) * VW)
                for ih in range(NIH):
                    nc.tensor.matmul(
                        o_ps[:, ih * FB:(ih + 1) * FB],
                        v_sb[nk][:, vcols],
                        e_sb[:, ih * FB:(ih + 1) * FB],
                        start=(nk == 0), stop=(nk == NKB - 1))

            def drain_head(q2, h):
                """aT copies + reciprocal of the denominator row."""
                t, odd = h // 2, h % 2
                cs = slice(q2 * FBS, (q2 + 1) * FBS)
                o_ps = o_tiles[(q2, h)]
                # lane-locked engines cannot shift partitions and DMA
                # cannot read PSUM, so shifting copies stage through SBUF
                stgd = stgp.tile([65, FBS], f32, name=f"sd{h}_{q2}",
                                 tag="stgd")
                if odd:
                    stg = stgp.tile([64, FBS], dt_proj, name=f"sg{h}_{q2}",
                                    tag="stg")
                    nc.vector.tensor_copy(stg[:], o_ps[0:64, :])
                    nc.sync.dma_start(aT[t][64:128, cs], stg[:])
                else:
                    nc.vector.tensor_copy(aT[t][0:64, cs], o_ps[0:64, :])
                sdb = stgp.tile([65, FBS], dt.bfloat16, name=f"sb{h}_{q2}",
                                tag="sdb")
                if (q2, h) == (NQ2 - 1, HPC - 1):
                    # last head: its reciprocal chain is the kernel tail, so
                    # use the 2-op ACT path 1/den = exp(-ln(den)) instead of
                    # the (longer but off-critical-path) DVE/DMA chain; Ln,
                    # Exp and Identity share one activation table
                    nc.scalar.activation(stgd[64:65, :], o_ps[64:65, :],
                                         AF.Ln)
                    nc.scalar.activation(sdb[64:65, :], stgd[64:65, :],
                                         AF.Exp, scale=-1.0)
                else:
                    nc.vector.tensor_copy(stgd[64:65, :], o_ps[64:65, :])
                    # den row -> denR [16, 64] block (rows h*32, col q2*64):
                    # reciprocal, DMA-reshape back to a row, cast to bf16
                    dblk = denR[h * 32:h * 32 + 16, q2 * 64:(q2 + 1) * 64]
                    nc.sync.dma_start(dblk, stgd[64:65, :])
                    nc.vector.reciprocal(dblk, dblk)
                    sdf = stgp.tile([65, FBS], f32, name=f"sf{h}_{q2}",
                                    tag="sdf")
                    nc.sync.dma_start(sdf[64:65, :], dblk)
                    nc.vector.tensor_copy(sdb[64:65, :], sdf[64:65, :])
                stgd_t[(q2, h)] = sdb

            def norm_half(q2, t, half):
                """PE-broadcast the 1/den rows of heads 2t,2t+1 and scale."""
                cols = slice(q2 * FBS + half * FB, q2 * FBS + (half + 1) * FB)
                bc = ps_pjp.tile([128, FB], f32, name="bc", tag="pj")
                for par in range(2):
                    sd = stgd_t[(q2, 2 * t + par)]
                    nc.tensor.matmul(
                        bc[:],
                        bcpat[64:65, par * 128:(par + 1) * 128],
                        sd[64:65, half * FB:(half + 1) * FB],
                        start=(par == 0), stop=(par == 1))
                nc.vector.tensor_mul(aT[t][:, cols], aT[t][:, cols], bc[:])
                nc.vector.tensor_scalar_add(aT[t][:, cols], aT[t][:, cols],
                                            vbv_sb[:, t:t + 1])

            def proj_blk(blk):
                bs = slice(blk * 128, (blk + 1) * 128)
                pj = ps_pjp.tile([128, FB], f32, name="pj", tag="pj")
                for f in range(2):
                    nc.tensor.matmul(pj[:], aT[f][:, bs], pjt_sb[f][:],
                                     start=(f == 0), stop=(f == 1))
                ob = osbp.tile([128, FB], f32, name="ob", tag="ob")
                nc.vector.tensor_copy(ob[:], pj[:])
                if blk >= N // 128 - 2:
                    # tail blocks: split across both queues to cut the drain
                    nc.gpsimd.dma_start(io["out"][blk * 128:blk * 128 + 64, :],
                                        ob[0:64, :])
                    nc.sync.dma_start(io["out"][blk * 128 + 64:
                                                (blk + 1) * 128, :],
                                      ob[64:128, :])
                else:
                    eng = nc.gpsimd if blk % 2 == 0 else nc.sync
                    eng.dma_start(io["out"][bs, :], ob[:])

            # fill work injected at (q2, h, nk) steps: normalizes once both
            # heads of a t-group drained (nk>=4: the reciprocal chain takes
            # ~4us after the drain pops at nk=0), stripe-0 projection spread
            # one block per two steps across stripe 1's first two windows
            fills = {
                (0, 2, 4): [lambda: norm_half(0, 0, 0)],
                (0, 2, 5): [lambda: norm_half(0, 0, 1)],
                (1, 0, 4): [lambda: norm_half(0, 1, 0)],
                (1, 0, 5): [lambda: norm_half(0, 1, 1)],
                (1, 2, 4): [lambda: norm_half(1, 0, 0)],
                (1, 2, 5): [lambda: norm_half(1, 0, 1)],
            }
            proj_steps = [(1, 0, 7), (1, 0, 9), (1, 0, 11), (1, 0, 13),
                          (1, 0, 15), (1, 1, 1), (1, 1, 3), (1, 1, 5)]
            for i, step in enumerate(proj_steps):
                fills.setdefault(step, []).append(lambda blk=i: proj_blk(blk))

            # attnV lags scores/exp via a queue: lag 3 across a head start
            # (so the previous head's o_ps drain — bufs=1 — never stalls the
            # PE), catching back up to lag 1 within the head
            pend_q = []

            NLAST = NKB // 2 - 1 if AV8 else NKB - 1

            def pop_pend():
                q2p, hp, nkp, e = pend_q.pop(0)
                attn_v(q2p, hp, nkp, e)
                if nkp == NLAST:
                    drain_head(q2p, hp)

            e_cur = None
            for q2 in range(NQ2):
                qbase = q2 * FBS
                for h in range(HPC):
                    for nk in range(NKB):
                        for f in fills.get((q2, h, nk), ()):
                            f()
                        ks = slice(nk * 128, (nk + 1) * 128)
                        s_ps = ps_sp.tile([128, FBS], f32, name="sps",
                                          tag="sps")
                        for ih in range(NIH):
                            hqs = slice(qbase + ih * FB, qbase + (ih + 1) * FB)
                            nc.tensor.matmul(s_ps[:, ih * FB:(ih + 1) * FB],
                                             KP[h][:, ks], QP[h][:, hqs],
                                             start=True, stop=True)
                        if AV8:
                            if nk % 2 == 0:
                                e_cur = esbp.tile([128, 2 * FBS], dt_av,
                                                  name="esb", tag="esb")
                            nc.scalar.activation(
                                e_cur[:, (nk % 2) * FBS:(nk % 2 + 1) * FBS],
                                s_ps[:], AF.Exp)
                            if nk % 2 == 1:
                                pend_q.append((q2, h, nk // 2, e_cur))
                                while len(pend_q) > (2 if pend_q[0][2] < 1
                                                     else 1):
                                    pop_pend()
                        else:
                            e_sb = esbp.tile([128, FBS], dt_attn, name="esb",
                                             tag="esb")
                            nc.scalar.activation(e_sb[:], s_ps[:], AF.Exp)
                            pend_q.append((q2, h, nk, e_sb))
                            while len(pend_q) > (3 if pend_q[0][2] <= 1
                                                 else 1):
                                pop_pend()
            while pend_q:
                pop_pend()
            norm_half(1, 1, 0)
            norm_half(1, 1, 1)
            for blk in range(FBS // 128, N // 128):
                proj_blk(blk)


def _build(cfg_key):
    from concourse import bacc, mybir, tile

    cfg = dict(cfg_key)
    dt = mybir.dt
    nc = bacc.Bacc("TRN2", target_bir_lowering=False, debug=False,
                   num_devices=8)
    _d = {"f32": dt.float32, "f32r": dt.float32r, "bf16": dt.bfloat16}
    dt_qkv = _d[cfg["qkv"]]
    dt_proj = _d[cfg["proj"]]
    dt_conv = dt.bfloat16 if cfg["attn"] == "bf16" else dt.float32
    shapes = {
        "xt": ([DIM, N], dt_qkv), "mt": ([256, N], dt_conv),
        "st": ([256, N], dt_conv),
        "wqk": ([DIM, 512], dt_qkv), "wv": ([DIM, 256], dt_qkv),
        "pjt": ([256, DIM], dt_proj),
        "mcw": ([128, 8], dt.float32),
        "scw": ([128, 8], dt.float32),
        "qkb": ([128, 4], dt.float32), "vbv": ([128, 2], dt.float32),
    }
    io = {}
    for name, (shape, dtt) in shapes.items():
        io[name] = nc.dram_tensor(name, shape, dtt,
                                  kind="ExternalInput").ap()
    io["out"] = nc.dram_tensor("out", [N, DIM], dt.float32,
                               kind="ExternalOutput").ap()
    with tile.TileContext(nc) as tc:
        _emit(tc, nc, io, cfg)
    nc.compile()
    return nc


def _get_program(cfg):
    key = tuple(sorted(cfg.items()))
    if key not in _CACHE:
        _CACHE[key] = _build(key)
    return _CACHE[key]


# ------------------------------------------------------------------ wrapper
def kernel(_cfg=None, _want_results=False, **inputs):
    from concourse.bass_utils import run_bass_kernel_spmd

    cfg = dict(_DEFAULT_CFG)
    if _cfg:
        cfg.update(_cfg)
    env_cfg = os.environ.get("BASSKERN_CFG")
    if env_cfg:  # e.g. "attn=f32r,qkv=f32r"
        for kv in env_cfg.split(","):
            k, v = kv.split("=")
            cfg[k] = v

    inputs = {k: np.asarray(v, dtype=np.float32) for k, v in inputs.items()}
    nc = _get_program(cfg)
    in_maps = [_host_prep(core, inputs, cfg) for core in range(8)]
    res = run_bass_kernel_spmd(nc, in_maps, list(range(8)))

    out = np.empty((B, N, DIM), np.float32)
    pb = inputs["proj_b"]
    for b in range(B):
        out[b] = res.results[2 * b]["out"] + res.results[2 * b + 1]["out"] + pb
    if _want_results:
        return out, res
    return out
